# revision 63
# baseline (speedup 1.0000x reference)
"""CS-TreeLSTM (BRANCH=16, DEPTH=4, IN=HID=512) on 8 Trainium2 NeuronCores.

Strategy (data-parallel over subtrees, per the sharding hint):
  - Each core owns 8192 leaves, 512 level-3, 32 level-2, 2 level-1 nodes.
  - Activations live transposed on-chip: [hid/in on partitions, nodes on free].
  - Gate matmuls run as f32r (fp32-reduced, ~tf32) at bf16 PE speed.
  - Sibling sums (h_sum, sum_k f_k*C_k) are grouped free-dim reduces on DVE.
  - The parent-x term of the forget gate is folded into the PE accumulation via
    an indicator-matrix matmul (fx in natural layout as stationary, S moving).
  - Level 1 runs with natural-layout gates (nodes on partitions) to keep N=512.
  - Level 0 (one root; children span all cores) is combined on the host from
    per-core H1/C1 outputs (the only cross-core communication, 8x8KB).

Built on bacc.Bacc so multi-semaphore waits are legalized into event
semaphores automatically (TRN2 allows one sync wait per instruction).
"""

import sys

sys.path.insert(0, "/opt/trn_rl_repo")

import numpy as np

BRANCH = 16
DEPTH = 4
IN = 512
HID = 512
NC_N = 8
SIZES = [BRANCH**d for d in range(DEPTH + 1)]  # [1,16,256,4096,65536]
OFFS = [0, 1, 17, 273, 4369, 69905]
XT_COLS = 8192 + 512 + 32 + 2  # 8738
C3_OFF = 8192
C2_OFF = 8192 + 512
N_CHUNK = 16

_CACHE = {}


def _build_nc(cfg=None):
    cfg = cfg or {}
    from concourse import bacc
    import concourse.mybir as mybir
    import concourse.tile as tile
    from concourse.masks import make_identity

    F32 = mybir.dt.float32
    F32R = mybir.dt.float32r
    ACTF = mybir.ActivationFunctionType
    AX = mybir.AxisListType
    OP = mybir.AluOpType

    nc = bacc.Bacc()

    xt = nc.declare_dram_parameter("xt", [IN, XT_COLS], F32, isOutput=False)
    wname = ["wix", "wih", "wox", "woh", "wux", "wuh", "wfx", "wfh"]
    wps = {n: nc.declare_dram_parameter(n, [IN, HID], F32, isOutput=False) for n in wname}
    bT = {g: nc.declare_dram_parameter("bT" + g, [128, 4], F32, isOutput=False) for g in "iouf"}
    bD = nc.declare_dram_parameter("bD", [128, 4, 8], F32, isOutput=False)
    S_p = nc.declare_dram_parameter("S", [128, 512], F32, isOutput=False)
    out_hc = nc.declare_dram_parameter("out_hc", [4, HID], F32, isOutput=True)

    def t_view(h):  # DRAM [512, n] -> [128 part, 4 ktile, n] view
        return h[:, :].rearrange("(t p) n -> p t n", p=128)

    from contextlib import ExitStack

    with tile.TileContext(nc) as tc, ExitStack() as ctx:
        consts = ctx.enter_context(tc.tile_pool(name="consts", bufs=1))
        stream = ctx.enter_context(tc.tile_pool(name="stream", bufs=cfg.get("stream", 5)))
        workA = ctx.enter_context(tc.tile_pool(name="workA", bufs=cfg.get("workA", 2)))
        workB = ctx.enter_context(tc.tile_pool(name="workB", bufs=cfg.get("workB", 2)))
        longp = ctx.enter_context(tc.tile_pool(name="longp", bufs=1))
        psum = ctx.enter_context(tc.tile_pool(name="psum", bufs=cfg.get("psum", 4), space="PSUM"))
        psum_half = cfg.get("psum_half", False)

        # ---------------- constants / weights ----------------
        # DMA order is the startup critical path: first chunk's x, then the
        # i/o/u x-part weights, then fx3 inputs, then the rest. The h-part
        # weights (wih/woh/wuh) are only needed at level 3, so they ride the
        # leaf x stream pool late instead of holding SBUF all along.
        W = {}
        bTs = {}
        stream_tiles = {}

        def load_w(n):
            W[n] = consts.tile([128, 4, HID], F32R, tag="w_" + n, name="w_" + n)
            nc.sync.dma_start(out=W[n][:, :, :], in_=t_view(wps[n]).bitcast(F32R))

        def load_chunk(c):
            t = stream.tile([128, 4, 512], F32R, tag="xt_c", name=f"xt_c{c}")
            nc.sync.dma_start(
                out=t[:, :, :], in_=t_view(xt)[:, :, c * 512 : (c + 1) * 512].bitcast(F32R)
            )
            stream_tiles[c] = t
            return t

        # chunk 0 and wix stream per k-tile, interleaved, so the first gate's
        # k-waves start as soon as each 0.5 MB pair lands
        t0 = stream.tile([128, 4, 512], F32R, tag="xt_c", name="xt_c0")
        stream_tiles[0] = t0
        W["wix"] = consts.tile([128, 4, HID], F32R, tag="w_wix", name="w_wix")
        for k in range(4):
            nc.sync.dma_start(out=t0[:, k, :], in_=t_view(xt)[:, k, 0:512].bitcast(F32R))
            nc.sync.dma_start(out=W["wix"][:, k, :], in_=t_view(wps["wix"])[:, k, :].bitcast(F32R))
        for g in "iouf":
            bTs[g] = consts.tile([128, 4], F32, tag="bT" + g, name="bT" + g)
            nc.sync.dma_start(out=bTs[g][:, :], in_=bT[g][:, :])
        load_w("wox")
        load_w("wux")
        xt3 = stream.tile([128, 4, 512], F32R, tag="xt_c", name="xt3")
        nc.sync.dma_start(out=xt3[:, :, :], in_=t_view(xt)[:, :, C3_OFF : C3_OFF + 512].bitcast(F32R))
        load_w("wfx")
        load_chunk(1)
        load_w("wfh")
        S_sb = consts.tile([128, 512], F32R, tag="S")
        nc.sync.dma_start(out=S_sb[:, :], in_=S_p[:, :].bitcast(F32R))
        bD_sb = consts.tile([128, 4, 8], F32, tag="bD")
        nc.sync.dma_start(out=bD_sb[:, :, :], in_=bD[:, :, :])
        ident = consts.tile([128, 128], F32, tag="ident")
        make_identity(nc, ident[:, :])

        # resident x tail (level-2/1 x)
        xt_tail = consts.tile([128, 4, 34], F32R, tag="xt_tail")
        nc.sync.dma_start(out=xt_tail[:, :, :], in_=t_view(xt)[:, :, C2_OFF : C2_OFF + 34].bitcast(F32R))

        # persistent accumulators
        hsum3T = longp.tile([128, 4, 512], F32R, tag="hsum3T")
        fcsum3T = longp.tile([128, 4, 512], F32, tag="fcsum3T")
        fx3_nat = longp.tile([128, 4, 512], F32R, tag="fx3_nat")

        def gate_T(g, rhs_x, rhs_h, n, aug=None):
            """Transposed-layout gate accumulation into a fresh psum tile.
            g in {i,o,u}: pre[:,m,:n] = sum_k WgxT[k,m].T @ rhs_x[k]
                          (+ sum_k WghT[k,m].T @ rhs_h[k]) (+ aug)
            g == "fh":    pre[:,m,:n] = sum_k WfhT[k,m].T @ rhs_x[k] (+ aug)"""
            pa = psum.tile([128, 2, 512], F32, tag="ps", name="ps_a")
            pb2 = psum.tile([128, 2, 512], F32, tag="ps", name="ps_b")
            ps = (pa, pb2)

            def slot(m):
                return ps[m // 2][:, m % 2, :n]

            if rhs_h is not None:
                # all x-part matmuls first (independent), then the h-part wave:
                # the first h matmul may wait on hsum, covered by 16 x matmuls
                for m in range(4):
                    ms = slice(m * 128, (m + 1) * 128)
                    for k in range(4):
                        nc.tensor.matmul(
                            slot(m), W["w" + g + "x"][:, k, ms], rhs_x[:, k, :],
                            start=(k == 0), stop=False,
                        )
                for m in range(4):
                    ms = slice(m * 128, (m + 1) * 128)
                    for k in range(4):
                        nc.tensor.matmul(
                            slot(m), W["w" + g + "h"][:, k, ms], rhs_h[:, k, :],
                            start=False, stop=(k == 3),
                        )
                return ps
            for m in range(4):
                ms = slice(m * 128, (m + 1) * 128)
                if g == "fh":
                    seq = [(W["wfh"], rhs_x, k) for k in range(4)]
                else:
                    seq = [(W["w" + g + "x"], rhs_x, k) for k in range(4)]
                naug = 1 if aug is not None else 0
                for idx, (wt, rhs, k) in enumerate(seq):
                    nc.tensor.matmul(
                        slot(m), wt[:, k, ms], rhs[:, k, :],
                        start=(idx == 0), stop=(naug == 0 and idx == len(seq) - 1),
                    )
                if naug:
                    la, ra, base = aug(ms)
                    nc.tensor.matmul(
                        slot(m), la, ra, start=False, stop=True,
                        tile_position=(base, 0),
                    )
            return ps

        def evac(ps, n, act, bias_g, out_sb):
            for m in range(4):
                b = 0.0 if bias_g is None else bTs[bias_g][:, m : m + 1]
                nc.scalar.activation(out_sb[:, m, :n], ps[m // 2][:, m % 2, :n], act, bias=b)

        LOWP = "f32r rounding for downstream matmul"

        def fx3_compute():
            # fx3_nat = x3 @ WfxT (natural layout)
            psx = (psum.tile([128, 2, 512], F32, tag="ps", name="ps_a"),
                   psum.tile([128, 2, 512], F32, tag="ps", name="ps_b"))
            for pb in range(4):
                for k in range(4):
                    nc.tensor.matmul(
                        psx[pb // 2][:, pb % 2, :], xt3[:, k, pb * 128 : (pb + 1) * 128],
                        W["wfx"][:, k, :], start=(k == 0), stop=(k == 3),
                    )
            for pb in range(4):
                nc.scalar.activation(fx3_nat[:, pb, :], psx[pb // 2][:, pb % 2, :], ACTF.Copy)

        # ---------------- leaf phase ----------------
        # The f-gate matmuls for chunk c need C(c) (a DVE product of ACT
        # outputs); running them one chunk behind keeps PE from stalling on
        # the ACT/DVE tail of the current chunk.
        def leaf_fpath(c, C_prev):
            b, pt = 32 * (c % 4), c // 4

            def aug3(ms, b=b, pt=pt):
                return fx3_nat[b : b + 32, pt, ms], S_sb[b : b + 32, :], b

            ps_f = gate_T("fh", C_prev, None, 512, aug=aug3)
            f_sb = workB.tile([128, 4, 512], F32, tag="Ug")
            evac(ps_f, 512, ACTF.Sigmoid, "f", f_sb)
            fC_sb = workB.tile([128, 4, 512], F32, tag="H")
            for m in range(4):
                nc.vector.tensor_mul(fC_sb[:, m, :], f_sb[:, m, :], C_prev[:, m, :].bitcast(F32))
            for m in range(4):
                nc.vector.tensor_reduce(
                    fcsum3T[:, m, 32 * c : 32 * c + 32],
                    fC_sb[:, m, :].rearrange("p (g w) -> p g w", w=16),
                    axis=AX.X, op=OP.add,
                )

        def leaf_hpath(c, C_prev, o_prev):
            tC_sb = workA.tile([128, 4, 512], F32, tag="A")
            H_sb = workB.tile([128, 4, 512], F32, tag="H")
            for m in range(4):
                nc.scalar.activation(tC_sb[:, m, :], C_prev[:, m, :].bitcast(F32), ACTF.Tanh)
            for m in range(4):
                nc.vector.tensor_mul(H_sb[:, m, :], o_prev[:, m, :], tC_sb[:, m, :])
            with nc.allow_low_precision(LOWP):
                for m in range(4):
                    nc.vector.tensor_reduce(
                        hsum3T[:, m, 32 * c : 32 * c + 32],
                        H_sb[:, m, :].rearrange("p (g w) -> p g w", w=16),
                        axis=AX.X, op=OP.add,
                    )

        pipe = None  # (chunk index, C_sb, o_sb)
        for c in range(N_CHUNK):
            xt_c = stream_tiles[c] if c in stream_tiles else load_chunk(c)

            if c == 0:
                # k-outer so each arriving (x, wix) k-tile pair is consumed
                ps_i = (psum.tile([128, 2, 512], F32, tag="ps", name="ps_a"),
                        psum.tile([128, 2, 512], F32, tag="ps", name="ps_b"))
                for k in range(4):
                    for m in range(4):
                        nc.tensor.matmul(
                            ps_i[m // 2][:, m % 2, :], W["wix"][:, k, m * 128 : (m + 1) * 128],
                            xt_c[:, k, :], start=(k == 0), stop=(k == 3),
                        )
            else:
                ps_i = gate_T("i", xt_c, None, 512)
            i_sb = workA.tile([128, 4, 512], F32, tag="A")
            evac(ps_i, 512, ACTF.Sigmoid, "i", i_sb)

            ps_o = gate_T("o", xt_c, None, 512)
            o_sb = workB.tile([128, 4, 512], F32, tag="B")
            evac(ps_o, 512, ACTF.Sigmoid, "o", o_sb)

            ps_u = gate_T("u", xt_c, None, 512)
            u_sb = workB.tile([128, 4, 512], F32, tag="Ug")
            evac(ps_u, 512, ACTF.Tanh, "u", u_sb)

            C_sb = workA.tile([128, 4, 512], F32R, tag="C")
            for m in range(4):
                nc.vector.tensor_mul(C_sb[:, m, :], i_sb[:, m, :], u_sb[:, m, :])

            if c == 0:
                fx3_compute()
            if pipe is not None and not cfg.get("no_fpath"):
                leaf_fpath(pipe[0], pipe[1])

            if not cfg.get("no_hpath"):
                leaf_hpath(c, C_sb, o_sb)
            pipe = (c, C_sb, o_sb)

        leaf_fpath(pipe[0], pipe[1])

        # late-loaded h-part weights (ride the stream pool slots)
        for n in ("wih", "woh", "wuh"):
            W[n] = stream.tile([128, 4, HID], F32R, tag="xt_c", name="w_" + n)
            nc.sync.dma_start(out=W[n][:, :, :], in_=t_view(wps[n]).bitcast(F32R))

        # ---------------- level 3 (512 nodes, transposed) ----------------
        ps3 = gate_T("i", xt3, hsum3T, 512)
        i3 = workA.tile([128, 4, 512], F32, tag="A")
        evac(ps3, 512, ACTF.Sigmoid, "i", i3)
        ps3 = gate_T("o", xt3, hsum3T, 512)
        o3 = workB.tile([128, 4, 512], F32, tag="B")
        evac(ps3, 512, ACTF.Sigmoid, "o", o3)
        ps3 = gate_T("u", xt3, hsum3T, 512)
        u3 = workB.tile([128, 4, 512], F32, tag="Ug")
        evac(ps3, 512, ACTF.Tanh, "u", u3)

        # fx2_nat [32,512] and fx1_nat [2,512] (independent of the leaf/L3
        # dataflow; emitted here so PE stays busy while C3 is produced)
        ps = psum.tile([128, 2, 512], F32, tag="ps", name="ps_a")
        for k in range(4):
            nc.tensor.matmul(
                ps[0:32, 0, :], xt_tail[:, k, 0:32], W["wfx"][:, k, :], start=(k == 0), stop=(k == 3)
            )
        for k in range(4):
            nc.tensor.matmul(
                ps[0:2, 1, :], xt_tail[:, k, 32:34], W["wfx"][:, k, :], start=(k == 0), stop=(k == 3)
            )
        fx2_nat = longp.tile([128, 512], F32R, tag="fx2_nat")
        nc.scalar.activation(fx2_nat[0:32, :], ps[0:32, 0, :], ACTF.Copy)
        fx1_nat = longp.tile([128, 512], F32R, tag="fx1_nat")
        nc.scalar.activation(fx1_nat[0:2, :], ps[0:2, 1, :], ACTF.Copy)

        C3 = workA.tile([128, 4, 512], F32R, tag="C")
        iu3 = workB.tile([128, 4, 512], F32, tag="H")
        for m in range(4):
            nc.vector.tensor_mul(iu3[:, m, :], i3[:, m, :], u3[:, m, :])
            with nc.allow_low_precision(LOWP):
                nc.vector.tensor_add(C3[:, m, :], iu3[:, m, :], fcsum3T[:, m, :])
        tC3 = workA.tile([128, 4, 512], F32, tag="A")
        H3 = workB.tile([128, 4, 512], F32, tag="H")
        for m in range(4):
            nc.scalar.activation(tC3[:, m, :], C3[:, m, :].bitcast(F32), ACTF.Tanh)
        for m in range(4):
            nc.vector.tensor_mul(H3[:, m, :], o3[:, m, :], tC3[:, m, :])

        # ---------------- f-path to level 2 ----------------
        # fx2/fx1 were computed right after the level-3 gates (independent
        # work that fills the PE gap while C3 is produced).

        def aug2(ms):
            return fx2_nat[0:32, ms], S_sb[0:32, :], 0

        ps_f2 = gate_T("fh", C3, None, 512, aug=aug2)
        f2 = workB.tile([128, 4, 512], F32, tag="Ug")
        evac(ps_f2, 512, ACTF.Sigmoid, "f", f2)
        fC2 = workB.tile([128, 4, 512], F32, tag="B")
        for m in range(4):
            nc.vector.tensor_mul(fC2[:, m, :], f2[:, m, :], C3[:, m, :].bitcast(F32))

        hsum2T = longp.tile([128, 4, 32], F32R, tag="hsum2T")
        fcsum2T = longp.tile([128, 4, 32], F32, tag="fcsum2T")
        with nc.allow_low_precision(LOWP):
            nc.vector.tensor_reduce(
                hsum2T[:, :, :],
                H3[:, :, :].rearrange("p t (g w) -> p t g w", w=16),
                axis=AX.X, op=OP.add,
            )
        nc.vector.tensor_reduce(
            fcsum2T[:, :, :],
            fC2[:, :, :].rearrange("p t (g w) -> p t g w", w=16),
            axis=AX.X, op=OP.add,
        )

        # ---------------- level 2 (32 nodes, transposed) ----------------
        x2v = xt_tail[:, :, 0:32]
        ps2 = gate_T("i", x2v, hsum2T, 32)
        i2 = longp.tile([128, 4, 32], F32, tag="s_i")
        evac(ps2, 32, ACTF.Sigmoid, "i", i2)
        ps2 = gate_T("o", x2v, hsum2T, 32)
        o2 = longp.tile([128, 4, 32], F32, tag="s_o")
        evac(ps2, 32, ACTF.Sigmoid, "o", o2)
        ps2 = gate_T("u", x2v, hsum2T, 32)
        u2 = longp.tile([128, 4, 32], F32, tag="s_u")
        evac(ps2, 32, ACTF.Tanh, "u", u2)

        # level-1 gate x-parts hoisted here (independent of the level-2
        # ladder) into held-open psum groups; h-parts + bias land later
        l1T1 = psum.tile([128, 2, 512], F32, tag="ps", name="ps_a")
        l1T2 = psum.tile([128, 2, 512], F32, tag="ps", name="ps_b")
        l1slot = {"i": l1T1[0:2, 0, :], "o": l1T1[0:2, 1, :], "u": l1T2[0:2, 0, :]}
        for g in "iou":
            for k in range(4):
                nc.tensor.matmul(
                    l1slot[g], xt_tail[:, k, 32:34], W["w" + g + "x"][:, k, :],
                    start=(k == 0), stop=False,
                )

        C2 = longp.tile([128, 4, 32], F32R, tag="C2")
        iu2 = longp.tile([128, 4, 32], F32, tag="s_t")
        nc.vector.tensor_mul(iu2[:, :, :], i2[:, :, :], u2[:, :, :])
        with nc.allow_low_precision(LOWP):
            nc.vector.tensor_add(C2[:, :, :], iu2[:, :, :], fcsum2T[:, :, :])
        tC2 = longp.tile([128, 4, 32], F32, tag="s_t2")
        nc.scalar.activation(tC2[:, :, :], C2[:, :, :].bitcast(F32), ACTF.Tanh)
        H2 = longp.tile([128, 4, 32], F32, tag="s_h")
        nc.vector.tensor_mul(H2[:, :, :], o2[:, :, :], tC2[:, :, :])

        # ---------------- f-path to level 1 ----------------

        def aug1(ms):
            return fx1_nat[0:2, ms], S_sb[0:2, 0:32], 0

        ps_f1 = gate_T("fh", C2, None, 32, aug=aug1)
        f1 = longp.tile([128, 4, 32], F32, tag="s_f1")
        evac(ps_f1, 32, ACTF.Sigmoid, "f", f1)
        fC1 = longp.tile([128, 4, 32], F32, tag="s_fc1")
        nc.vector.tensor_mul(fC1[:, :, :], f1[:, :, :], C2[:, :, :].bitcast(F32))

        hsum1T = longp.tile([128, 4, 2], F32R, tag="hsum1T")
        fcsum1T = longp.tile([128, 4, 2], F32, tag="fcsum1T")
        with nc.allow_low_precision(LOWP):
            nc.vector.tensor_reduce(
                hsum1T[:, :, :],
                H2[:, :, :].rearrange("p t (g w) -> p t g w", w=16),
                axis=AX.X, op=OP.add,
            )
        nc.vector.tensor_reduce(
            fcsum1T[:, :, :],
            fC1[:, :, :].rearrange("p t (g w) -> p t g w", w=16),
            axis=AX.X, op=OP.add,
        )

        # ---------------- level 1 (2 nodes, natural-layout gates) ----------------
        # [2,512] scratch tensors share two tiles via free-dim offsets (DVE
        # two-input ops require equal base partitions, so all sit at rows 0:2).
        buf_a = longp.tile([128, 1024], F32, tag="buf_a")
        buf_b = longp.tile([128, 1024], F32, tag="buf_b")
        g1 = {}
        for gi, (g, act) in enumerate(
            (("i", ACTF.Sigmoid), ("o", ACTF.Sigmoid), ("u", ACTF.Tanh))
        ):
            for k in range(4):
                nc.tensor.matmul(
                    l1slot[g], hsum1T[:, k, :], W["w" + g + "h"][:, k, :],
                    start=False, stop=False,
                )
            # bias: transpose a duplicated bias column into both node rows
            bidx = "iouf".index(g)
            for m in range(4):
                nc.tensor.matmul(
                    l1slot[g][:, m * 128 : (m + 1) * 128],
                    bD_sb[:, m, 2 * bidx : 2 * bidx + 2], ident[:, :],
                    is_transpose=True, start=False, stop=(m == 3),
                )
            g1[g] = (buf_a[0:2, 0:512], buf_a[0:2, 512:1024], buf_b[0:2, 512:1024])[gi]
            nc.scalar.activation(g1[g], l1slot[g], act)

        # fcsum1 to natural layout via PE transpose
        ps_t = psum.tile([128, 2, 512], F32, tag="ps", name="ps_a")
        for t in range(4):
            nc.tensor.transpose(ps_t[0:2, 0, t * 128 : (t + 1) * 128], fcsum1T[:, t, :], ident[:, :])

        iu1 = buf_b[0:2, 0:512]
        nc.vector.tensor_mul(iu1, g1["i"], g1["u"])
        C1 = buf_b[0:2, 512:1024]
        nc.vector.tensor_add(C1, iu1, ps_t[0:2, 0, :])
        tC1 = buf_a[0:2, 0:512]
        nc.scalar.activation(tC1, C1, ACTF.Tanh)
        H1 = buf_b[0:2, 0:512]
        nc.vector.tensor_mul(H1, g1["o"], tC1)

        nc.sync.dma_start(out=out_hc[0:2, :], in_=H1)
        nc.sync.dma_start(out=out_hc[2:4, :], in_=C1)

    nc.finalize()
    return nc


def _np_sigmoid(v):
    return 1.0 / (1.0 + np.exp(-v))


def _host_prep(x, wi_w, wo_w, wu_w, wf_w, wi_b, wo_b, wu_b, wf_b):
    xt_full = np.ascontiguousarray(x.T)  # [512, 69905]

    def wT(w, part):
        return np.ascontiguousarray(w[:, :512].T if part == "x" else w[:, 512:].T)

    common = {
        "wix": wT(wi_w, "x"), "wih": wT(wi_w, "h"),
        "wox": wT(wo_w, "x"), "woh": wT(wo_w, "h"),
        "wux": wT(wu_w, "x"), "wuh": wT(wu_w, "h"),
        "wfx": wT(wf_w, "x"), "wfh": wT(wf_w, "h"),
        "bTi": np.ascontiguousarray(np.asarray(wi_b).reshape(4, 128).T),
        "bTo": np.ascontiguousarray(np.asarray(wo_b).reshape(4, 128).T),
        "bTu": np.ascontiguousarray(np.asarray(wu_b).reshape(4, 128).T),
        "bTf": np.ascontiguousarray(np.asarray(wf_b).reshape(4, 128).T),
        "bD": np.ascontiguousarray(
            np.stack([np.asarray(b).reshape(4, 128).T for b in (wi_b, wo_b, wu_b, wf_b)], axis=2)
            .repeat(2, axis=2).reshape(128, 4, 8)
        ),
        "S": (np.arange(512)[None, :] // 16 == (np.arange(128) % 32)[:, None]).astype(np.float32),
    }
    in_maps = []
    for c in range(NC_N):
        xt_c = np.concatenate(
            [
                xt_full[:, OFFS[4] + 8192 * c : OFFS[4] + 8192 * (c + 1)],
                xt_full[:, OFFS[3] + 512 * c : OFFS[3] + 512 * (c + 1)],
                xt_full[:, OFFS[2] + 32 * c : OFFS[2] + 32 * (c + 1)],
                xt_full[:, OFFS[1] + 2 * c : OFFS[1] + 2 * (c + 1)],
            ],
            axis=1,
        )
        in_maps.append({"xt": np.ascontiguousarray(xt_c), **common})
    return in_maps


def _host_finish(x, H1_all, C1_all, wi_w, wi_b, wf_w, wf_b, wo_w, wo_b, wu_w, wu_b):
    """Level 0 (root): its 16 children are the level-1 nodes across cores."""
    f8 = np.float64
    x0 = np.asarray(x[0], f8)
    H1 = np.asarray(H1_all, f8)
    C1 = np.asarray(C1_all, f8)
    hsum0 = H1.sum(0)
    f0 = _np_sigmoid(
        x0 @ np.asarray(wf_w, f8)[:, :512].T + C1 @ np.asarray(wf_w, f8)[:, 512:].T + np.asarray(wf_b, f8)
    )
    fcsum0 = (f0 * C1).sum(0)
    xh0 = np.concatenate([x0, hsum0])
    i0 = _np_sigmoid(xh0 @ np.asarray(wi_w, f8).T + np.asarray(wi_b, f8))
    o0 = _np_sigmoid(xh0 @ np.asarray(wo_w, f8).T + np.asarray(wo_b, f8))
    u0 = np.tanh(xh0 @ np.asarray(wu_w, f8).T + np.asarray(wu_b, f8))
    C0 = i0 * u0 + fcsum0
    H0 = o0 * np.tanh(C0)
    return H0.astype(np.float32), C0.astype(np.float32)


def _run(in_maps, trace=False):
    from concourse.bass_utils import run_bass_kernel_spmd

    if "nc" not in _CACHE:
        _CACHE["nc"] = _build_nc()
    return run_bass_kernel_spmd(_CACHE["nc"], in_maps, list(range(NC_N)), trace=trace)


def kernel(x, wi_w, wi_b, wf_w, wf_b, wo_w, wo_b, wu_w, wu_b, _trace=False):
    x = np.asarray(x, np.float32)
    in_maps = _host_prep(x, wi_w, wo_w, wu_w, wf_w, wi_b, wo_b, wu_b, wf_b)
    res = _run(in_maps, trace=_trace)
    _CACHE["last_results"] = res
    H1_all = np.concatenate([res.results[c]["out_hc"][0:2] for c in range(NC_N)])
    C1_all = np.concatenate([res.results[c]["out_hc"][2:4] for c in range(NC_N)])
    H0, C0 = _host_finish(x, H1_all, C1_all, wi_w, wi_b, wf_w, wf_b, wo_w, wo_b, wu_w, wu_b)
    return H0, C0



# revision 67
# speedup vs baseline: 1.0347x; 1.0347x over previous
"""CS-TreeLSTM (BRANCH=16, DEPTH=4, IN=HID=512) on 8 Trainium2 NeuronCores.

Strategy (data-parallel over subtrees, per the sharding hint):
  - Each core owns 8192 leaves, 512 level-3, 32 level-2, 2 level-1 nodes.
  - Activations live transposed on-chip: [hid/in on partitions, nodes on free].
  - Gate matmuls run as f32r (fp32-reduced, ~tf32) at bf16 PE speed.
  - Sibling sums (h_sum, sum_k f_k*C_k) are grouped free-dim reduces on DVE.
  - The parent-x term of the forget gate is folded into the PE accumulation via
    an indicator-matrix matmul (fx in natural layout as stationary, S moving).
  - Level 1 runs with natural-layout gates (nodes on partitions) to keep N=512.
  - Level 0 (one root; children span all cores) is combined on the host from
    per-core H1/C1 outputs (the only cross-core communication, 8x8KB).

Built on bacc.Bacc so multi-semaphore waits are legalized into event
semaphores automatically (TRN2 allows one sync wait per instruction).
"""

import sys

sys.path.insert(0, "/opt/trn_rl_repo")

import numpy as np

BRANCH = 16
DEPTH = 4
IN = 512
HID = 512
NC_N = 8
SIZES = [BRANCH**d for d in range(DEPTH + 1)]  # [1,16,256,4096,65536]
OFFS = [0, 1, 17, 273, 4369, 69905]
XT_COLS = 8192 + 512 + 32 + 2  # 8738
C3_OFF = 8192
C2_OFF = 8192 + 512
N_CHUNK = 16

_CACHE = {}


def _build_nc(cfg=None):
    cfg = cfg or {}
    from concourse import bacc
    import concourse.mybir as mybir
    import concourse.tile as tile
    from concourse.masks import make_identity

    F32 = mybir.dt.float32
    F32R = mybir.dt.float32r
    ACTF = mybir.ActivationFunctionType
    AX = mybir.AxisListType
    OP = mybir.AluOpType

    nc = bacc.Bacc()

    xt = nc.declare_dram_parameter("xt", [IN, XT_COLS], F32, isOutput=False)
    F8 = mybir.dt.float8e4
    DR = mybir.MatmulPerfMode.DoubleRow
    xt8 = nc.declare_dram_parameter("xt8", [IN, XT_COLS], F8, isOutput=False)
    xt8l = nc.declare_dram_parameter("xt8l", [IN, XT_COLS], F8, isOutput=False)
    wox8h = nc.declare_dram_parameter("wox8h", [IN, HID], F8, isOutput=False)
    wox8d = nc.declare_dram_parameter("wox8d", [IN, HID], F8, isOutput=False)
    wname = ["wix", "wih", "wox", "woh", "wux", "wuh", "wfx", "wfh"]
    wps = {n: nc.declare_dram_parameter(n, [IN, HID], F32, isOutput=False) for n in wname}
    bT = {g: nc.declare_dram_parameter("bT" + g, [128, 4], F32, isOutput=False) for g in "iouf"}
    bD = nc.declare_dram_parameter("bD", [128, 4, 8], F32, isOutput=False)
    S_p = nc.declare_dram_parameter("S", [128, 512], F32, isOutput=False)
    out_hc = nc.declare_dram_parameter("out_hc", [4, HID], F32, isOutput=True)

    def t_view(h):  # DRAM [512, n] -> [128 part, 4 ktile, n] view
        return h[:, :].rearrange("(t p) n -> p t n", p=128)

    from contextlib import ExitStack

    with tile.TileContext(nc) as tc, ExitStack() as ctx:
        consts = ctx.enter_context(tc.tile_pool(name="consts", bufs=1))
        stream = ctx.enter_context(tc.tile_pool(name="stream", bufs=cfg.get("stream", 3)))
        stream8 = ctx.enter_context(tc.tile_pool(name="stream8", bufs=2))
        workA = ctx.enter_context(tc.tile_pool(name="workA", bufs=cfg.get("workA", 2)))
        workB = ctx.enter_context(tc.tile_pool(name="workB", bufs=cfg.get("workB", 2)))
        longp = ctx.enter_context(tc.tile_pool(name="longp", bufs=1))
        psum = ctx.enter_context(tc.tile_pool(name="psum", bufs=cfg.get("psum", 4), space="PSUM"))
        psum_half = cfg.get("psum_half", False)

        # ---------------- constants / weights ----------------
        # DMA order is the startup critical path: first chunk's x, then the
        # i/o/u x-part weights, then fx3 inputs, then the rest. The h-part
        # weights (wih/woh/wuh) are only needed at level 3, so they ride the
        # leaf x stream pool late instead of holding SBUF all along.
        W = {}
        bTs = {}
        stream_tiles = {}

        def load_w(n):
            W[n] = consts.tile([128, 4, HID], F32R, tag="w_" + n, name="w_" + n)
            nc.sync.dma_start(out=W[n][:, :, :], in_=t_view(wps[n]).bitcast(F32R))

        w8 = {}
        for nm, prm in (("h", wox8h), ("d", wox8d)):
            w8[nm] = consts.tile([128, 4, HID], F8, tag="wox8" + nm, name="wox8" + nm)
            nc.sync.dma_start(out=w8[nm][:, :, :], in_=t_view(prm))
        stream8_tiles = {}

        def load_chunk(c):
            t = stream.tile([128, 4, 512], F32R, tag="xt_c", name=f"xt_c{c}")
            nc.sync.dma_start(
                out=t[:, :, :], in_=t_view(xt)[:, :, c * 512 : (c + 1) * 512].bitcast(F32R)
            )
            stream_tiles[c] = t
            t8 = stream8.tile([128, 4, 512], F8, tag="x8", name=f"x8_{c}")
            nc.sync.dma_start(out=t8[:, :, :], in_=t_view(xt8)[:, :, c * 512 : (c + 1) * 512])
            tl8 = stream8.tile([128, 4, 512], F8, tag="xl8", name=f"xl8_{c}")
            nc.sync.dma_start(out=tl8[:, :, :], in_=t_view(xt8l)[:, :, c * 512 : (c + 1) * 512])
            stream8_tiles[c] = (t8, tl8)
            return t

        # chunk 0 and wix stream per k-tile, interleaved, so the first gate's
        # k-waves start as soon as each 0.5 MB pair lands
        t0 = stream.tile([128, 4, 512], F32R, tag="xt_c", name="xt_c0")
        stream_tiles[0] = t0
        t0_8 = stream8.tile([128, 4, 512], F8, tag="x8", name="x8_0")
        nc.sync.dma_start(out=t0_8[:, :, :], in_=t_view(xt8)[:, :, 0:512])
        t0_l8 = stream8.tile([128, 4, 512], F8, tag="xl8", name="xl8_0")
        nc.sync.dma_start(out=t0_l8[:, :, :], in_=t_view(xt8l)[:, :, 0:512])
        stream8_tiles[0] = (t0_8, t0_l8)
        W["wix"] = consts.tile([128, 4, HID], F32R, tag="w_wix", name="w_wix")
        for k in range(4):
            nc.sync.dma_start(out=t0[:, k, :], in_=t_view(xt)[:, k, 0:512].bitcast(F32R))
            nc.sync.dma_start(out=W["wix"][:, k, :], in_=t_view(wps["wix"])[:, k, :].bitcast(F32R))
        for g in "iouf":
            bTs[g] = consts.tile([128, 4], F32, tag="bT" + g, name="bT" + g)
            nc.sync.dma_start(out=bTs[g][:, :], in_=bT[g][:, :])
        load_w("wox")
        load_w("wux")
        xt3 = stream.tile([128, 4, 512], F32R, tag="xt_c", name="xt3")
        nc.sync.dma_start(out=xt3[:, :, :], in_=t_view(xt)[:, :, C3_OFF : C3_OFF + 512].bitcast(F32R))
        load_w("wfx")
        load_chunk(1)
        load_w("wfh")
        S_sb = consts.tile([128, 512], F32R, tag="S")
        nc.sync.dma_start(out=S_sb[:, :], in_=S_p[:, :].bitcast(F32R))
        bD_sb = consts.tile([128, 4, 8], F32, tag="bD")
        nc.sync.dma_start(out=bD_sb[:, :, :], in_=bD[:, :, :])
        ident = consts.tile([128, 128], F32, tag="ident")
        make_identity(nc, ident[:, :])

        # resident x tail (level-2/1 x)
        xt_tail = consts.tile([128, 4, 34], F32R, tag="xt_tail")
        nc.sync.dma_start(out=xt_tail[:, :, :], in_=t_view(xt)[:, :, C2_OFF : C2_OFF + 34].bitcast(F32R))

        # persistent accumulators
        hsum3T = longp.tile([128, 4, 512], F32R, tag="hsum3T")
        fcsum3T = longp.tile([128, 4, 512], F32, tag="fcsum3T")
        fx3_nat = longp.tile([128, 4, 512], F32R, tag="fx3_nat")

        def gate_T(g, rhs_x, rhs_h, n, aug=None):
            """Transposed-layout gate accumulation into a fresh psum tile.
            g in {i,o,u}: pre[:,m,:n] = sum_k WgxT[k,m].T @ rhs_x[k]
                          (+ sum_k WghT[k,m].T @ rhs_h[k]) (+ aug)
            g == "fh":    pre[:,m,:n] = sum_k WfhT[k,m].T @ rhs_x[k] (+ aug)"""
            pa = psum.tile([128, 2, 512], F32, tag="ps", name="ps_a")
            pb2 = psum.tile([128, 2, 512], F32, tag="ps", name="ps_b")
            ps = (pa, pb2)

            def slot(m):
                return ps[m // 2][:, m % 2, :n]

            if rhs_h is not None:
                # all x-part matmuls first (independent), then the h-part wave:
                # the first h matmul may wait on hsum, covered by 16 x matmuls
                for m in range(4):
                    ms = slice(m * 128, (m + 1) * 128)
                    for k in range(4):
                        nc.tensor.matmul(
                            slot(m), W["w" + g + "x"][:, k, ms], rhs_x[:, k, :],
                            start=(k == 0), stop=False,
                        )
                for m in range(4):
                    ms = slice(m * 128, (m + 1) * 128)
                    for k in range(4):
                        nc.tensor.matmul(
                            slot(m), W["w" + g + "h"][:, k, ms], rhs_h[:, k, :],
                            start=False, stop=(k == 3),
                        )
                return ps
            for m in range(4):
                ms = slice(m * 128, (m + 1) * 128)
                if g == "fh":
                    seq = [(W["wfh"], rhs_x, k) for k in range(4)]
                else:
                    seq = [(W["w" + g + "x"], rhs_x, k) for k in range(4)]
                naug = 1 if aug is not None else 0
                for idx, (wt, rhs, k) in enumerate(seq):
                    nc.tensor.matmul(
                        slot(m), wt[:, k, ms], rhs[:, k, :],
                        start=(idx == 0), stop=(naug == 0 and idx == len(seq) - 1),
                    )
                if naug:
                    la, ra, base = aug(ms)
                    nc.tensor.matmul(
                        slot(m), la, ra, start=False, stop=True,
                        tile_position=(base, 0),
                    )
            return ps

        def evac(ps, n, act, bias_g, out_sb, scale=1.0):
            for m in range(4):
                b = 0.0 if bias_g is None else bTs[bias_g][:, m : m + 1]
                nc.scalar.activation(out_sb[:, m, :n], ps[m // 2][:, m % 2, :n], act, bias=b, scale=scale)

        def o_gate_dr(x8_c, xl8_c):
            pa = psum.tile([128, 2, 512], F32, tag="ps", name="ps_a")
            pb2 = psum.tile([128, 2, 512], F32, tag="ps", name="ps_b")
            ps = (pa, pb2)
            for m in range(4):
                ms = slice(m * 128, (m + 1) * 128)
                for wv, rr in ((w8["h"], x8_c), (w8["d"], xl8_c)):
                    for kp in range(2):
                        nc.tensor.matmul(
                            ps[m // 2][:, m % 2, :], wv[:, 2 * kp : 2 * kp + 2, ms],
                            rr[:, 2 * kp : 2 * kp + 2, :],
                            start=(wv is w8["h"] and kp == 0),
                            stop=(wv is w8["d"] and kp == 1), perf_mode=DR,
                        )
            return ps

        LOWP = "f32r rounding for downstream matmul"

        def fx3_compute():
            # fx3_nat = x3 @ WfxT (natural layout)
            psx = (psum.tile([128, 2, 512], F32, tag="ps", name="ps_a"),
                   psum.tile([128, 2, 512], F32, tag="ps", name="ps_b"))
            for pb in range(4):
                for k in range(4):
                    nc.tensor.matmul(
                        psx[pb // 2][:, pb % 2, :], xt3[:, k, pb * 128 : (pb + 1) * 128],
                        W["wfx"][:, k, :], start=(k == 0), stop=(k == 3),
                    )
            for pb in range(4):
                nc.scalar.activation(fx3_nat[:, pb, :], psx[pb // 2][:, pb % 2, :], ACTF.Copy)

        # ---------------- leaf phase ----------------
        # The f-gate matmuls for chunk c need C(c) (a DVE product of ACT
        # outputs); running them one chunk behind keeps PE from stalling on
        # the ACT/DVE tail of the current chunk.
        def leaf_fpath(c, C_prev):
            b, pt = 32 * (c % 4), c // 4

            def aug3(ms, b=b, pt=pt):
                return fx3_nat[b : b + 32, pt, ms], S_sb[b : b + 32, :], b

            ps_f = gate_T("fh", C_prev, None, 512, aug=aug3)
            f_sb = workB.tile([128, 4, 512], F32, tag="Ug")
            evac(ps_f, 512, ACTF.Sigmoid, "f", f_sb)
            fC_sb = workB.tile([128, 4, 512], F32, tag="H")
            for m in range(4):
                nc.vector.tensor_mul(fC_sb[:, m, :], f_sb[:, m, :], C_prev[:, m, :].bitcast(F32))
            for m in range(4):
                nc.vector.tensor_reduce(
                    fcsum3T[:, m, 32 * c : 32 * c + 32],
                    fC_sb[:, m, :].rearrange("p (g w) -> p g w", w=16),
                    axis=AX.X, op=OP.add,
                )

        def leaf_hpath(c, C_prev, o_prev):
            tC_sb = workA.tile([128, 4, 512], F32, tag="A")
            H_sb = workB.tile([128, 4, 512], F32, tag="H")
            for m in range(4):
                nc.scalar.activation(tC_sb[:, m, :], C_prev[:, m, :].bitcast(F32), ACTF.Tanh)
            for m in range(4):
                nc.vector.tensor_mul(H_sb[:, m, :], o_prev[:, m, :], tC_sb[:, m, :])
            with nc.allow_low_precision(LOWP):
                for m in range(4):
                    nc.vector.tensor_reduce(
                        hsum3T[:, m, 32 * c : 32 * c + 32],
                        H_sb[:, m, :].rearrange("p (g w) -> p g w", w=16),
                        axis=AX.X, op=OP.add,
                    )

        pipe = None  # (chunk index, C_sb, o_sb)
        for c in range(N_CHUNK):
            xt_c = stream_tiles[c] if c in stream_tiles else load_chunk(c)

            if c == 0:
                # k-outer so each arriving (x, wix) k-tile pair is consumed
                ps_i = (psum.tile([128, 2, 512], F32, tag="ps", name="ps_a"),
                        psum.tile([128, 2, 512], F32, tag="ps", name="ps_b"))
                for k in range(4):
                    for m in range(4):
                        nc.tensor.matmul(
                            ps_i[m // 2][:, m % 2, :], W["wix"][:, k, m * 128 : (m + 1) * 128],
                            xt_c[:, k, :], start=(k == 0), stop=(k == 3),
                        )
            else:
                ps_i = gate_T("i", xt_c, None, 512)
            i_sb = workA.tile([128, 4, 512], F32, tag="A")
            evac(ps_i, 512, ACTF.Sigmoid, "i", i_sb)

            x8_c, xl8_c = stream8_tiles.pop(c)
            ps_o = o_gate_dr(x8_c, xl8_c)
            o_sb = workB.tile([128, 4, 512], F32, tag="B")
            evac(ps_o, 512, ACTF.Sigmoid, "o", o_sb, scale=1 / 4096.0)

            ps_u = gate_T("u", xt_c, None, 512)
            u_sb = workB.tile([128, 4, 512], F32, tag="Ug")
            evac(ps_u, 512, ACTF.Tanh, "u", u_sb)

            C_sb = workA.tile([128, 4, 512], F32R, tag="C")
            for m in range(4):
                nc.vector.tensor_mul(C_sb[:, m, :], i_sb[:, m, :], u_sb[:, m, :])

            if c == 0:
                fx3_compute()
            if pipe is not None and not cfg.get("no_fpath"):
                leaf_fpath(pipe[0], pipe[1])

            if not cfg.get("no_hpath"):
                leaf_hpath(c, C_sb, o_sb)
            pipe = (c, C_sb, o_sb)

        leaf_fpath(pipe[0], pipe[1])

        # late-loaded h-part weights (ride the stream pool slots)
        for n in ("wih", "woh", "wuh"):
            W[n] = stream.tile([128, 4, HID], F32R, tag="xt_c", name="w_" + n)
            nc.sync.dma_start(out=W[n][:, :, :], in_=t_view(wps[n]).bitcast(F32R))

        # ---------------- level 3 (512 nodes, transposed) ----------------
        ps3 = gate_T("i", xt3, hsum3T, 512)
        i3 = workA.tile([128, 4, 512], F32, tag="A")
        evac(ps3, 512, ACTF.Sigmoid, "i", i3)
        ps3 = gate_T("o", xt3, hsum3T, 512)
        o3 = workB.tile([128, 4, 512], F32, tag="B")
        evac(ps3, 512, ACTF.Sigmoid, "o", o3)
        ps3 = gate_T("u", xt3, hsum3T, 512)
        u3 = workB.tile([128, 4, 512], F32, tag="Ug")
        evac(ps3, 512, ACTF.Tanh, "u", u3)

        # fx2_nat [32,512] and fx1_nat [2,512] (independent of the leaf/L3
        # dataflow; emitted here so PE stays busy while C3 is produced)
        ps = psum.tile([128, 2, 512], F32, tag="ps", name="ps_a")
        for k in range(4):
            nc.tensor.matmul(
                ps[0:32, 0, :], xt_tail[:, k, 0:32], W["wfx"][:, k, :], start=(k == 0), stop=(k == 3)
            )
        for k in range(4):
            nc.tensor.matmul(
                ps[0:2, 1, :], xt_tail[:, k, 32:34], W["wfx"][:, k, :], start=(k == 0), stop=(k == 3)
            )
        fx2_nat = longp.tile([128, 512], F32R, tag="fx2_nat")
        nc.scalar.activation(fx2_nat[0:32, :], ps[0:32, 0, :], ACTF.Copy)
        fx1_nat = longp.tile([128, 512], F32R, tag="fx1_nat")
        nc.scalar.activation(fx1_nat[0:2, :], ps[0:2, 1, :], ACTF.Copy)

        C3 = workA.tile([128, 4, 512], F32R, tag="C")
        iu3 = workB.tile([128, 4, 512], F32, tag="H")
        for m in range(4):
            nc.vector.tensor_mul(iu3[:, m, :], i3[:, m, :], u3[:, m, :])
            with nc.allow_low_precision(LOWP):
                nc.vector.tensor_add(C3[:, m, :], iu3[:, m, :], fcsum3T[:, m, :])
        tC3 = workA.tile([128, 4, 512], F32, tag="A")
        H3 = workB.tile([128, 4, 512], F32, tag="H")
        for m in range(4):
            nc.scalar.activation(tC3[:, m, :], C3[:, m, :].bitcast(F32), ACTF.Tanh)
        for m in range(4):
            nc.vector.tensor_mul(H3[:, m, :], o3[:, m, :], tC3[:, m, :])

        # ---------------- f-path to level 2 ----------------
        # fx2/fx1 were computed right after the level-3 gates (independent
        # work that fills the PE gap while C3 is produced).

        def aug2(ms):
            return fx2_nat[0:32, ms], S_sb[0:32, :], 0

        ps_f2 = gate_T("fh", C3, None, 512, aug=aug2)
        f2 = workB.tile([128, 4, 512], F32, tag="Ug")
        evac(ps_f2, 512, ACTF.Sigmoid, "f", f2)
        fC2 = workB.tile([128, 4, 512], F32, tag="B")
        for m in range(4):
            nc.vector.tensor_mul(fC2[:, m, :], f2[:, m, :], C3[:, m, :].bitcast(F32))

        hsum2T = longp.tile([128, 4, 32], F32R, tag="hsum2T")
        fcsum2T = longp.tile([128, 4, 32], F32, tag="fcsum2T")
        with nc.allow_low_precision(LOWP):
            nc.vector.tensor_reduce(
                hsum2T[:, :, :],
                H3[:, :, :].rearrange("p t (g w) -> p t g w", w=16),
                axis=AX.X, op=OP.add,
            )
        nc.vector.tensor_reduce(
            fcsum2T[:, :, :],
            fC2[:, :, :].rearrange("p t (g w) -> p t g w", w=16),
            axis=AX.X, op=OP.add,
        )

        # ---------------- level 2 (32 nodes, transposed) ----------------
        x2v = xt_tail[:, :, 0:32]
        ps2 = gate_T("i", x2v, hsum2T, 32)
        i2 = longp.tile([128, 4, 32], F32, tag="s_i")
        evac(ps2, 32, ACTF.Sigmoid, "i", i2)
        ps2 = gate_T("o", x2v, hsum2T, 32)
        o2 = longp.tile([128, 4, 32], F32, tag="s_o")
        evac(ps2, 32, ACTF.Sigmoid, "o", o2)
        ps2 = gate_T("u", x2v, hsum2T, 32)
        u2 = longp.tile([128, 4, 32], F32, tag="s_u")
        evac(ps2, 32, ACTF.Tanh, "u", u2)

        # level-1 gate x-parts hoisted here (independent of the level-2
        # ladder) into held-open psum groups; h-parts + bias land later
        l1T1 = psum.tile([128, 2, 512], F32, tag="ps", name="ps_a")
        l1T2 = psum.tile([128, 2, 512], F32, tag="ps", name="ps_b")
        l1slot = {"i": l1T1[0:2, 0, :], "o": l1T1[0:2, 1, :], "u": l1T2[0:2, 0, :]}
        for g in "iou":
            for k in range(4):
                nc.tensor.matmul(
                    l1slot[g], xt_tail[:, k, 32:34], W["w" + g + "x"][:, k, :],
                    start=(k == 0), stop=False,
                )

        C2 = longp.tile([128, 4, 32], F32R, tag="C2")
        iu2 = longp.tile([128, 4, 32], F32, tag="s_t")
        nc.vector.tensor_mul(iu2[:, :, :], i2[:, :, :], u2[:, :, :])
        with nc.allow_low_precision(LOWP):
            nc.vector.tensor_add(C2[:, :, :], iu2[:, :, :], fcsum2T[:, :, :])
        tC2 = longp.tile([128, 4, 32], F32, tag="s_t2")
        nc.scalar.activation(tC2[:, :, :], C2[:, :, :].bitcast(F32), ACTF.Tanh)
        H2 = longp.tile([128, 4, 32], F32, tag="s_h")
        nc.vector.tensor_mul(H2[:, :, :], o2[:, :, :], tC2[:, :, :])

        # ---------------- f-path to level 1 ----------------

        def aug1(ms):
            return fx1_nat[0:2, ms], S_sb[0:2, 0:32], 0

        ps_f1 = gate_T("fh", C2, None, 32, aug=aug1)
        f1 = longp.tile([128, 4, 32], F32, tag="s_f1")
        evac(ps_f1, 32, ACTF.Sigmoid, "f", f1)
        fC1 = longp.tile([128, 4, 32], F32, tag="s_fc1")
        nc.vector.tensor_mul(fC1[:, :, :], f1[:, :, :], C2[:, :, :].bitcast(F32))

        hsum1T = longp.tile([128, 4, 2], F32R, tag="hsum1T")
        fcsum1T = longp.tile([128, 4, 2], F32, tag="fcsum1T")
        with nc.allow_low_precision(LOWP):
            nc.vector.tensor_reduce(
                hsum1T[:, :, :],
                H2[:, :, :].rearrange("p t (g w) -> p t g w", w=16),
                axis=AX.X, op=OP.add,
            )
        nc.vector.tensor_reduce(
            fcsum1T[:, :, :],
            fC1[:, :, :].rearrange("p t (g w) -> p t g w", w=16),
            axis=AX.X, op=OP.add,
        )

        # ---------------- level 1 (2 nodes, natural-layout gates) ----------------
        # [2,512] scratch tensors share two tiles via free-dim offsets (DVE
        # two-input ops require equal base partitions, so all sit at rows 0:2).
        buf_a = longp.tile([128, 1024], F32, tag="buf_a")
        buf_b = longp.tile([128, 1024], F32, tag="buf_b")
        g1 = {}
        for gi, (g, act) in enumerate(
            (("i", ACTF.Sigmoid), ("o", ACTF.Sigmoid), ("u", ACTF.Tanh))
        ):
            for k in range(4):
                nc.tensor.matmul(
                    l1slot[g], hsum1T[:, k, :], W["w" + g + "h"][:, k, :],
                    start=False, stop=False,
                )
            # bias: transpose a duplicated bias column into both node rows
            bidx = "iouf".index(g)
            for m in range(4):
                nc.tensor.matmul(
                    l1slot[g][:, m * 128 : (m + 1) * 128],
                    bD_sb[:, m, 2 * bidx : 2 * bidx + 2], ident[:, :],
                    is_transpose=True, start=False, stop=(m == 3),
                )
            g1[g] = (buf_a[0:2, 0:512], buf_a[0:2, 512:1024], buf_b[0:2, 512:1024])[gi]
            nc.scalar.activation(g1[g], l1slot[g], act)

        # fcsum1 to natural layout via PE transpose
        ps_t = psum.tile([128, 2, 512], F32, tag="ps", name="ps_a")
        for t in range(4):
            nc.tensor.transpose(ps_t[0:2, 0, t * 128 : (t + 1) * 128], fcsum1T[:, t, :], ident[:, :])

        iu1 = buf_b[0:2, 0:512]
        nc.vector.tensor_mul(iu1, g1["i"], g1["u"])
        C1 = buf_b[0:2, 512:1024]
        nc.vector.tensor_add(C1, iu1, ps_t[0:2, 0, :])
        tC1 = buf_a[0:2, 0:512]
        nc.scalar.activation(tC1, C1, ACTF.Tanh)
        H1 = buf_b[0:2, 0:512]
        nc.vector.tensor_mul(H1, g1["o"], tC1)

        nc.sync.dma_start(out=out_hc[0:2, :], in_=H1)
        nc.sync.dma_start(out=out_hc[2:4, :], in_=C1)

    nc.finalize()
    return nc


def _np_sigmoid(v):
    return 1.0 / (1.0 + np.exp(-v))


def _host_prep(x, wi_w, wo_w, wu_w, wf_w, wi_b, wo_b, wu_b, wf_b):
    import ml_dtypes

    E4 = ml_dtypes.float8_e4m3
    xt_full = np.ascontiguousarray(x.T)  # [512, 69905]
    xt8_full = (xt_full * 16.0).astype(E4)
    xt8l_full = xt_full.astype(E4)  # x at scale 1: moving for the weight-residual wave

    def wT(w, part):
        return np.ascontiguousarray(w[:, :512].T if part == "x" else w[:, 512:].T)

    common = {
        "wix": wT(wi_w, "x"), "wih": wT(wi_w, "h"),
        "wox": wT(wo_w, "x"), "woh": wT(wo_w, "h"),
        "wux": wT(wu_w, "x"), "wuh": wT(wu_w, "h"),
        "wfx": wT(wf_w, "x"), "wfh": wT(wf_w, "h"),
        "bTi": np.ascontiguousarray(np.asarray(wi_b).reshape(4, 128).T),
        "bTo": np.ascontiguousarray(np.asarray(wo_b).reshape(4, 128).T),
        "bTu": np.ascontiguousarray(np.asarray(wu_b).reshape(4, 128).T),
        "bTf": np.ascontiguousarray(np.asarray(wf_b).reshape(4, 128).T),
        "bD": np.ascontiguousarray(
            np.stack([np.asarray(b).reshape(4, 128).T for b in (wi_b, wo_b, wu_b, wf_b)], axis=2)
            .repeat(2, axis=2).reshape(128, 4, 8)
        ),
        "S": (np.arange(512)[None, :] // 16 == (np.arange(128) % 32)[:, None]).astype(np.float32),
        "wox8h": (np.ascontiguousarray(wo_w[:, :512].T) * 256.0).astype(E4),
        "wox8d": (16.0 * (np.ascontiguousarray(wo_w[:, :512].T) * 256.0
                          - (np.ascontiguousarray(wo_w[:, :512].T) * 256.0).astype(E4).astype(np.float32))).astype(E4),
    }
    in_maps = []
    for c in range(NC_N):
        def cols(full):
            return np.ascontiguousarray(np.concatenate(
                [
                    full[:, OFFS[4] + 8192 * c : OFFS[4] + 8192 * (c + 1)],
                    full[:, OFFS[3] + 512 * c : OFFS[3] + 512 * (c + 1)],
                    full[:, OFFS[2] + 32 * c : OFFS[2] + 32 * (c + 1)],
                    full[:, OFFS[1] + 2 * c : OFFS[1] + 2 * (c + 1)],
                ],
                axis=1,
            ))
        in_maps.append({"xt": cols(xt_full), "xt8": cols(xt8_full),
                        "xt8l": cols(xt8l_full), **common})
    return in_maps


def _host_finish(x, H1_all, C1_all, wi_w, wi_b, wf_w, wf_b, wo_w, wo_b, wu_w, wu_b):
    """Level 0 (root): its 16 children are the level-1 nodes across cores."""
    f8 = np.float64
    x0 = np.asarray(x[0], f8)
    H1 = np.asarray(H1_all, f8)
    C1 = np.asarray(C1_all, f8)
    hsum0 = H1.sum(0)
    f0 = _np_sigmoid(
        x0 @ np.asarray(wf_w, f8)[:, :512].T + C1 @ np.asarray(wf_w, f8)[:, 512:].T + np.asarray(wf_b, f8)
    )
    fcsum0 = (f0 * C1).sum(0)
    xh0 = np.concatenate([x0, hsum0])
    i0 = _np_sigmoid(xh0 @ np.asarray(wi_w, f8).T + np.asarray(wi_b, f8))
    o0 = _np_sigmoid(xh0 @ np.asarray(wo_w, f8).T + np.asarray(wo_b, f8))
    u0 = np.tanh(xh0 @ np.asarray(wu_w, f8).T + np.asarray(wu_b, f8))
    C0 = i0 * u0 + fcsum0
    H0 = o0 * np.tanh(C0)
    return H0.astype(np.float32), C0.astype(np.float32)


def _run(in_maps, trace=False):
    from concourse.bass_utils import run_bass_kernel_spmd

    if "nc" not in _CACHE:
        _CACHE["nc"] = _build_nc()
    return run_bass_kernel_spmd(_CACHE["nc"], in_maps, list(range(NC_N)), trace=trace)


def kernel(x, wi_w, wi_b, wf_w, wf_b, wo_w, wo_b, wu_w, wu_b, _trace=False):
    x = np.asarray(x, np.float32)
    in_maps = _host_prep(x, wi_w, wo_w, wu_w, wf_w, wi_b, wo_b, wu_b, wf_b)
    res = _run(in_maps, trace=_trace)
    _CACHE["last_results"] = res
    H1_all = np.concatenate([res.results[c]["out_hc"][0:2] for c in range(NC_N)])
    C1_all = np.concatenate([res.results[c]["out_hc"][2:4] for c in range(NC_N)])
    H0, C0 = _host_finish(x, H1_all, C1_all, wi_w, wi_b, wf_w, wf_b, wo_w, wo_b, wu_w, wu_b)
    return H0, C0



# revision 68
# speedup vs baseline: 1.0498x; 1.0146x over previous
"""CS-TreeLSTM (BRANCH=16, DEPTH=4, IN=HID=512) on 8 Trainium2 NeuronCores.

Strategy (data-parallel over subtrees, per the sharding hint):
  - Each core owns 8192 leaves, 512 level-3, 32 level-2, 2 level-1 nodes.
  - Activations live transposed on-chip: [hid/in on partitions, nodes on free].
  - Gate matmuls run as fp8-e4m3 DoubleRow (256-deep contraction per
    instruction at 0.5 cycles/row).  Weight quantization error is systematic
    across nodes and amplifies ~8x per tree level through the 16-child sums,
    so every weight is sent as a hi+lo fp8 PAIR (wl = fp8(16*(s*w - wh)));
    each gate runs a hi wave and a residual wave, giving ~1.3e-3 effective
    weight precision (2nd order in fp8 eps) at half the f32r PE cost.
    Activation (x, C, hsum) quantization is iid across nodes and cancels in
    the sibling sums, so single fp8 is enough for i/o/f; the u gate (tanh
    slope 1, feeds C directly) also gets an x-residual wave.
  - Scales: x8=16x, x8d=x, xl8=16*(16x-x8); Wx hi=256w; Wh hi=4096w;
    every i/o/u psum = 4096*pre, one merged ACT evac with scale 1/4096.
  - Gate biases ride tiny DoubleRow aug matmuls (hi slice x ones=16, lo
    residual slice x ones=1), making evacs single merged instructions (ACT is
    the bottleneck engine).
  - Forget gates: psum = Wfh_hi x C8(16C) + Wfh_lo x C8d(C) + f32r
    indicator-matrix aug carrying s*(Wfx x_parent + b_f) (computed once per
    level from an exact psum, weight pair included).
  - Sibling sums: h-path via bf16 pairwise-add tree (2x DVE mode) ->
    bf16 -> fp8 hi/lo; fcsum via plain reduce with f32 output.
  - Level 3 runs as two 256-node halves, the first interleaved at leaf chunk
    8 so its PE/DVE work hides under leaf-phase ACT; level 2 likewise as two
    16-node halves (chunk 10 / tail).
  - Level 0 (root; children span all cores) is combined on the host from
    per-core H1/C1 outputs (the only cross-core communication, 8x8KB).
"""

import sys

sys.path.insert(0, "/opt/trn_rl_repo")

import numpy as np

BRANCH = 16
DEPTH = 4
IN = 512
HID = 512
NC_N = 8
SIZES = [BRANCH**d for d in range(DEPTH + 1)]  # [1,16,256,4096,65536]
OFFS = [0, 1, 17, 273, 4369, 69905]
XT_COLS = 8192 + 512 + 32 + 2  # 8738
C3_OFF = 8192
C2_OFF = 8192 + 512
N_CHUNK = 16

SX = 16.0     # fp8 x hi scale
SWX = 256.0   # fp8 x-part / fh weight hi scale
SWH = 4096.0  # fp8 h-part weight hi scale
SPS = 4096.0  # i/o/u psum scale

_CACHE = {}

WNAMES = ["wix", "wih", "wox", "woh", "wux", "wuh", "wfx", "wfh"]


def _build_nc(cfg=None):
    cfg = cfg or {}
    from concourse import bacc
    import concourse.mybir as mybir
    import concourse.tile as tile
    from concourse.masks import make_identity

    F32 = mybir.dt.float32
    F32R = mybir.dt.float32r
    BF16 = mybir.dt.float16  # fp16: 10-bit mantissa, same DVE speed as bf16
    F8 = mybir.dt.float8e4
    ACTF = mybir.ActivationFunctionType
    AX = mybir.AxisListType
    OP = mybir.AluOpType
    DR = mybir.MatmulPerfMode.DoubleRow

    nc = bacc.Bacc()

    xt8 = nc.declare_dram_parameter("xt8", [IN, XT_COLS], F8, isOutput=False)
    xt8d = nc.declare_dram_parameter("xt8d", [IN, XT_COLS], F8, isOutput=False)
    xt8l = nc.declare_dram_parameter("xt8l", [IN, XT_COLS], F8, isOutput=False)
    wps = {}
    for n in WNAMES:
        wps[n + "_h"] = nc.declare_dram_parameter(n + "_h", [IN, HID], F8, isOutput=False)
        wps[n + "_l"] = nc.declare_dram_parameter(n + "_l", [IN, HID], F8, isOutput=False)
    for n in ("wux_d", "wix_d", "wox_d", "wuh_d", "wfh_d", "wfx_d"):
        wps[n] = nc.declare_dram_parameter(n, [IN, HID], F8, isOutput=False)
    bias8p = {g: nc.declare_dram_parameter("bias8" + g, [1, 1024], F8, isOutput=False) for g in "iou"}
    ones8p = nc.declare_dram_parameter("ones8", [1, 1024], F8, isOutput=False)
    b4096p = nc.declare_dram_parameter("b4096", [1, HID], F32, isOutput=False)
    S_p = nc.declare_dram_parameter("S", [128, 512], F32, isOutput=False)
    onescolp = nc.declare_dram_parameter("onescol", [1, 128], F32, isOutput=False)
    out_hc = nc.declare_dram_parameter("out_hc", [4, HID], F32, isOutput=True)
    dbg = {}
    if cfg.get("debug"):
        for nm, shp, dt_ in [
            ("dbg_hsum3", [128, 4, 512], BF16), ("dbg_fcsum3", [128, 4, 512], F32),
            ("dbg_C3", [128, 4, 512], BF16), ("dbg_fx3", [128, 4, 512], F32),
            ("dbg_hsum2", [128, 4, 32], BF16), ("dbg_fcsum2", [128, 4, 32], F32),
            ("dbg_C2", [128, 4, 32], BF16), ("dbg_hsum1", [128, 4, 2], BF16),
            ("dbg_fcsum1", [128, 4, 2], F32), ("dbg_C1", [128, 4, 2], F32),
            ("dbg_C0ch", [128, 4, 512], BF16),
        ]:
            dbg[nm] = nc.declare_dram_parameter(nm, shp, dt_, isOutput=True)

    def t_view(h):  # DRAM [512, n] -> [128 part, 4 ktile, n] view
        return h[:, :].rearrange("(t p) n -> p t n", p=128)

    from contextlib import ExitStack

    with tile.TileContext(nc) as tc, ExitStack() as ctx:
        consts = ctx.enter_context(tc.tile_pool(name="consts", bufs=1))
        stream = ctx.enter_context(tc.tile_pool(name="stream", bufs=cfg.get("stream", 2)))
        workA = ctx.enter_context(tc.tile_pool(name="workA", bufs=2))
        workB = ctx.enter_context(tc.tile_pool(name="workB", bufs=2))
        longp = ctx.enter_context(tc.tile_pool(name="longp", bufs=1))
        psum = ctx.enter_context(tc.tile_pool(name="psum", bufs=2, space="PSUM"))

        LOWP = "paired-fp8/bf16 pipeline, tolerance 2e-2"

        # ---------------- constants / weights ----------------
        bias8 = {}
        for g in "iou":
            bias8[g] = consts.tile([1, 4, 2, 128], F8, tag="bias8" + g, name="bias8" + g)
            nc.sync.dma_start(
                out=bias8[g][:, :, :, :],
                in_=bias8p[g][:, :].rearrange("a (m two f) -> a m two f", two=2, f=128),
            )
        ones8 = consts.tile([1, 2, 512], F8, tag="ones8")
        nc.sync.dma_start(out=ones8[:, :, :], in_=ones8p[:, :].rearrange("a (two f) -> a two f", two=2))

        stream_tiles = {}

        def load_chunk(c):
            th = stream.tile([128, 4, 512], F8, tag="xt_c", name=f"xt_c{c}")
            nc.sync.dma_start(out=th[:, :, :], in_=t_view(xt8)[:, :, c * 512 : (c + 1) * 512])
            td = stream.tile([128, 4, 512], F8, tag="xt_d", name=f"xt_d{c}")
            nc.sync.dma_start(out=td[:, :, :], in_=t_view(xt8d)[:, :, c * 512 : (c + 1) * 512])
            tl = stream.tile([128, 4, 512], F8, tag="xt_l", name=f"xt_l{c}")
            nc.sync.dma_start(out=tl[:, :, :], in_=t_view(xt8l)[:, :, c * 512 : (c + 1) * 512])
            stream_tiles[c] = (th, td, tl)
            return stream_tiles[c]

        W = {}

        def load_w(n):
            W[n] = consts.tile([128, 4, HID], F8, tag="w_" + n, name="w_" + n)
            nc.sync.dma_start(out=W[n][:, :, :], in_=t_view(wps[n]))

        load_chunk(0)
        load_w("wix_h")
        load_w("wix_l")
        load_w("wox_h")
        load_w("wox_l")
        load_w("wux_h")
        load_w("wux_l")
        load_w("wux_d")
        load_w("wix_d")
        load_w("wox_d")
        load_w("wuh_d")
        load_chunk(1)
        xt3 = {}
        for sfx, src in (("h", xt8), ("d", xt8d), ("l", xt8l)):
            xt3[sfx] = consts.tile([128, 4, 512], F8, tag="xt3" + sfx, name="xt3" + sfx)
            nc.sync.dma_start(out=xt3[sfx][:, :, :], in_=t_view(src)[:, :, C3_OFF : C3_OFF + 512])
        load_w("wfx_h")
        load_w("wfx_l")
        load_w("wfx_d")
        onescol = consts.tile([1, 128], F32R, tag="onescol")
        nc.sync.dma_start(out=onescol[:, :], in_=onescolp[:, :].bitcast(F32R))
        b4096 = consts.tile([1, 512], F32R, tag="b4096")
        nc.sync.dma_start(out=b4096[:, :], in_=b4096p[:, :].bitcast(F32R))
        S_sb = consts.tile([128, 512], F32R, tag="S")
        nc.sync.dma_start(out=S_sb[:, :], in_=S_p[:, :].bitcast(F32R))
        load_w("wfh_h")
        load_w("wfh_l")
        load_w("wfh_d")
        load_w("wih_h")
        load_w("wih_l")
        load_w("woh_h")
        load_w("woh_l")
        load_w("wuh_h")
        load_w("wuh_l")
        # tail x: [128,4,34] views for moving, clean-stride [128,4,32] for
        # fx stationaries (dual-fp8 Ldweights needs plane stride % 32 == 0)
        xtail = {}
        for sfx, src in (("h", xt8), ("d", xt8d), ("l", xt8l)):
            xtail[sfx] = consts.tile([128, 4, 34], F8, tag="xtail" + sfx, name="xtail" + sfx)
            nc.sync.dma_start(out=xtail[sfx][:, :, :], in_=t_view(src)[:, :, C2_OFF : C2_OFF + 34])
        xt2s = {}
        xt1s = {}
        for sfx, src in (("h", xt8), ("d", xt8d), ("l", xt8l)):
            xt2s[sfx] = consts.tile([128, 4, 32], F8, tag="xt2s" + sfx, name="xt2s" + sfx)
            nc.sync.dma_start(out=xt2s[sfx][:, :, :], in_=t_view(src)[:, :, C2_OFF : C2_OFF + 32])
            xt1s[sfx] = consts.tile([128, 4, 32], F8, tag="xt1s" + sfx, name="xt1s" + sfx)
            nc.sync.dma_start(out=xt1s[sfx][:, :, 0:2], in_=t_view(src)[:, :, C2_OFF + 32 : C2_OFF + 34])
        ident = consts.tile([128, 128], F32, tag="ident")
        make_identity(nc, ident[:, :])

        # persistent accumulators
        hsum3B = longp.tile([128, 4, 512], BF16, tag="hsum3B")
        hsum3 = {"h": longp.tile([128, 4, 512], F8, tag="hsum3h", name="hsum3h"),
                 "d": longp.tile([128, 4, 512], F8, tag="hsum3d", name="hsum3d")}
        hsum3res = longp.tile([128, 4, 512], F8, tag="hsum3res")
        hsT3 = longp.tile([128, 4, 32], BF16, tag="hsT3")
        fcsum3T = longp.tile([128, 4, 512], F32, tag="fcsum3T")
        fx3_nat = longp.tile([128, 4, 512], F32R, tag="fx3_nat")  # 4096*(fx3+bf)
        # [0:32, 0, :] = 2048*(fx2+bf); [0:2, 1, :] = 256*(fx1+bf)
        fx21_nat = longp.tile([128, 2, 512], F32R, tag="fx21_nat")

        def dr_gate(g, rhs, n=512, hpart=None, col0=0, hcol0=None):
            """i/o/u gate psum: 4096*(W x (+ W h) + b) via paired-fp8 DR waves.
            rhs = (hi, d, l) x tiles; hpart = (hi8, d8) hsum tiles.
            u gets a 3rd x wave (x residual, tanh slope 1 feeds C directly)."""
            if hcol0 is None:
                hcol0 = col0
            ps = psum.tile([128, 4, 512], F32, tag="ps")
            for m in range(4):
                nc.tensor.matmul(
                    ps[:, m, :n], bias8[g][0:1, m, :, :], ones8[:, :, :n],
                    start=True, stop=False, perf_mode=DR,
                )
            if g == "o":
                waves = [("wox_h", rhs[0], col0), ("wox_l", rhs[1], col0)]
            else:
                waves = [("w" + g + "x_h", rhs[0], col0), ("w" + g + "x_l", rhs[1], col0),
                         ("w" + g + "x_d", rhs[2], col0)]
            if hpart is not None:
                waves.append(("w" + g + "h_h", hpart[0], hcol0))
                waves.append(("w" + g + "h_l", hpart[1], hcol0))
                if g == "u":
                    waves.append(("wuh_d", hpart[2], hcol0))
            for wi, (wn, rr, c0) in enumerate(waves):
                last_wave = wi == len(waves) - 1
                for m in range(4):
                    ms = slice(m * 128, (m + 1) * 128)
                    for kp in range(2):
                        nc.tensor.matmul(
                            ps[:, m, :n], W[wn][:, 2 * kp : 2 * kp + 2, ms],
                            rr[:, 2 * kp : 2 * kp + 2, c0 : c0 + n],
                            start=False, stop=(last_wave and kp == 1), perf_mode=DR,
                        )
            return ps

        def evac(ps, out_sb, act, scale, n=512):
            nc.scalar.activation(out_sb[:, :, :n], ps[:, :, :n], act, scale=scale)

        def f_gate_dr(C8, C8d, C8l, fx_sb, fx_rows, fx_col, n=512):
            """forget gate psum: f32r indicator aug (fx+bias) + paired fp8 DR on C."""
            ps = psum.tile([128, 4, 512], F32, tag="ps")
            b0, b1 = fx_rows
            for m in range(4):
                ms = slice(m * 128, (m + 1) * 128)
                nc.tensor.matmul(
                    ps[:, m, :n], fx_sb[b0:b1, fx_col, ms], S_sb[b0:b1, :n],
                    start=True, stop=False, tile_position=(b0, 0),
                )
                for wn, rr in (("wfh_h", C8), ("wfh_l", C8d), ("wfh_d", C8l)):
                    for kp in range(2):
                        nc.tensor.matmul(
                            ps[:, m, :n], W[wn][:, 2 * kp : 2 * kp + 2, ms],
                            rr[:, 2 * kp : 2 * kp + 2, :n],
                            start=False, stop=(wn == "wfh_d" and kp == 1), perf_mode=DR,
                        )
            return ps

        def fx3_compute():
            # fx3_nat = 4096*(x3 @ WfxT + bf)  (natural layout, nodes on partitions)
            psx = psum.tile([128, 4, 512], F32, tag="ps")
            for pb in range(4):
                pbs = slice(pb * 128, (pb + 1) * 128)
                nc.tensor.matmul(psx[:, pb, :], onescol[:, :], b4096[:, :], start=True, stop=False)
                for sfx, wn in (("h", "wfx_h"), ("d", "wfx_l"), ("l", "wfx_d")):
                    for kp in range(2):
                        nc.tensor.matmul(
                            psx[:, pb, :], xt3[sfx][:, 2 * kp : 2 * kp + 2, pbs],
                            W[wn][:, 2 * kp : 2 * kp + 2, :],
                            start=False, stop=(sfx == "l" and kp == 1), perf_mode=DR,
                        )
            nc.scalar.activation(fx3_nat[:, :, :], psx[:, :, :], ACTF.Copy)

        def fx21_compute():
            ps = psum.tile([128, 4, 512], F32, tag="ps")
            nc.tensor.matmul(ps[0:32, 0, :], onescol[:, 0:32], b4096[:, :], start=True, stop=False)
            for sfx, wn in (("h", "wfx_h"), ("d", "wfx_l"), ("l", "wfx_d")):
                for kp in range(2):
                    nc.tensor.matmul(
                        ps[0:32, 0, :], xt2s[sfx][:, 2 * kp : 2 * kp + 2, :],
                        W[wn][:, 2 * kp : 2 * kp + 2, :],
                        start=False, stop=(sfx == "l" and kp == 1), perf_mode=DR,
                    )
            nc.tensor.matmul(ps[0:2, 1, :], onescol[:, 0:2], b4096[:, :], start=True, stop=False)
            for sfx, wn in (("h", "wfx_h"), ("d", "wfx_l"), ("l", "wfx_d")):
                for kp in range(2):
                    nc.tensor.matmul(
                        ps[0:2, 1, :], xt1s[sfx][:, 2 * kp : 2 * kp + 2, 0:2],
                        W[wn][:, 2 * kp : 2 * kp + 2, :],
                        start=False, stop=(sfx == "l" and kp == 1), perf_mode=DR,
                    )
            nc.scalar.activation(fx21_nat[0:32, 0, :], ps[0:32, 0, :], ACTF.Copy, scale=1 / 2.0)
            nc.scalar.activation(fx21_nat[0:2, 1, :], ps[0:2, 1, :], ACTF.Copy, scale=1 / 16.0)

        # ---------------- leaf phase ----------------
        def tree16(out_ap, src, pfx, g=32):
            # grouped sum of 16 along the free dim via bf16 pairwise adds (2x DVE mode)
            v = src[:, :, :].rearrange("p t (g w) -> p t g w", w=16)
            a1 = workB.tile([128, 4, g, 8], BF16, tag=f"tr1_{g}", name=pfx + "a1")
            a2 = workB.tile([128, 4, g, 4], BF16, tag=f"tr2_{g}", name=pfx + "a2")
            a3 = workB.tile([128, 4, g, 2], BF16, tag=f"tr3_{g}", name=pfx + "a3")
            nc.vector.tensor_add(a1[:, :, :, :], v[:, :, :, 0:8], v[:, :, :, 8:16])
            nc.vector.tensor_add(a2[:, :, :, :], a1[:, :, :, 0:4], a1[:, :, :, 4:8])
            nc.vector.tensor_add(a3[:, :, :, :], a2[:, :, :, 0:2], a2[:, :, :, 2:4])
            nc.vector.tensor_add(out_ap, a3[:, :, :, 0], a3[:, :, :, 1])

        def leaf_fpath(c, C_prev, C8_prev, C8d_prev, C8l_prev):
            b, pt = 32 * (c % 4), c // 4
            ps_f = f_gate_dr(C8_prev, C8d_prev, C8l_prev, fx3_nat, (b, b + 32), pt)
            f_bf = workB.tile([128, 4, 512], BF16, tag="Ug")
            evac(ps_f, f_bf, ACTF.Sigmoid, 1 / SPS)
            fC = workB.tile([128, 4, 512], BF16, tag="H")
            with nc.allow_low_precision(LOWP):
                nc.vector.tensor_mul(fC[:, :, :], f_bf[:, :, :], C_prev[:, :, :])
                nc.vector.tensor_reduce(
                    fcsum3T[:, :, 32 * c : 32 * c + 32],
                    fC[:, :, :].rearrange("p t (g w) -> p t g w", w=16),
                    axis=AX.X, op=OP.add,
                )

        def leaf_hpath(c, C_bf, o_bf):
            tC = workA.tile([128, 4, 512], BF16, tag="A")
            nc.scalar.activation(tC[:, :, :], C_bf[:, :, :], ACTF.Tanh)
            H = workB.tile([128, 4, 512], BF16, tag="H")
            cols = slice(32 * c, 32 * c + 32)
            with nc.allow_low_precision(LOWP):
                nc.vector.tensor_mul(H[:, :, :], o_bf[:, :, :], tC[:, :, :])
                tree16(hsum3B[:, :, cols], H, f"hs{c}")
                nc.vector.tensor_scalar_mul(hsum3["h"][:, :, cols], hsum3B[:, :, cols], 1.0)
                nc.vector.tensor_scalar_mul(hsum3["d"][:, :, cols], hsum3B[:, :, cols], 1 / 16.0)
                nc.vector.tensor_sub(hsT3[:, :, :], hsum3B[:, :, cols], hsum3["h"][:, :, cols])
                nc.vector.tensor_scalar_mul(hsum3res[:, :, cols], hsT3[:, :, :], 16.0)

        # ---------------- level 3 (512 nodes, two 256-node halves) ----------------
        C3_bf = longp.tile([128, 4, 512], BF16, tag="C3_bf")
        C8_3 = longp.tile([128, 4, 512], F8, tag="C8_3")
        C8_3d = longp.tile([128, 4, 512], F8, tag="C8_3d")
        C3res = longp.tile([128, 4, 512], F8, tag="C3res")
        hsum2B = longp.tile([128, 4, 32], BF16, tag="hsum2B")
        hsum2 = {"h": longp.tile([128, 4, 32], F8, tag="hsum2h", name="hsum2h"),
                 "d": longp.tile([128, 4, 32], F8, tag="hsum2d", name="hsum2d")}
        hsum2res = longp.tile([128, 4, 32], F8, tag="hsum2res")
        hsT2 = longp.tile([128, 4, 16], BF16, tag="hsT2")
        fcsum2T = longp.tile([128, 4, 32], F32, tag="fcsum2T")

        def l3_half(h):
            sl = slice(256 * h, 256 * h + 256)
            g16 = slice(16 * h, 16 * h + 16)
            hp = (hsum3["h"], hsum3["d"], hsum3res)
            x3t = (xt3["h"], xt3["d"], xt3["l"])
            ps3 = dr_gate("i", x3t, n=256, hpart=hp, col0=256 * h)
            i3 = workA.tile([128, 4, 256], BF16, tag="A3", name=f"i3_{h}")
            evac(ps3, i3, ACTF.Sigmoid, 1 / SPS, n=256)
            ps3 = dr_gate("o", x3t, n=256, hpart=hp, col0=256 * h)
            o3 = workB.tile([128, 4, 256], BF16, tag="B3", name=f"o3_{h}")
            evac(ps3, o3, ACTF.Sigmoid, 1 / SPS, n=256)
            ps3 = dr_gate("u", x3t, n=256, hpart=hp, col0=256 * h)
            u3 = workB.tile([128, 4, 256], BF16, tag="U3", name=f"u3_{h}")
            evac(ps3, u3, ACTF.Tanh, 1 / SPS, n=256)

            iu3 = workB.tile([128, 4, 256], BF16, tag="H3", name=f"iu3_{h}")
            C3t = workB.tile([128, 4, 256], BF16, tag="B3", name=f"C3t_{h}")
            with nc.allow_low_precision(LOWP):
                nc.vector.tensor_mul(iu3[:, :, :], i3[:, :, :], u3[:, :, :])
                nc.vector.tensor_add(C3_bf[:, :, sl], iu3[:, :, :], fcsum3T[:, :, sl])
                nc.vector.tensor_scalar_mul(C8_3[:, :, sl], C3_bf[:, :, sl], 8.0)
                nc.gpsimd.tensor_scalar_mul(C8_3d[:, :, sl], C3_bf[:, :, sl], 0.5)
                # C3res = 16*(8*C3 - C8_3): f2's C residual wave
                nc.vector.scalar_tensor_tensor(
                    C3t[:, :, :], C3_bf[:, :, sl], 8.0, C8_3[:, :, sl],
                    op0=OP.mult, op1=OP.subtract,
                )
                nc.gpsimd.tensor_scalar_mul(C3res[:, :, sl], C3t[:, :, :], 16.0)
            tC3 = workA.tile([128, 4, 256], BF16, tag="A3", name=f"tC3_{h}")
            nc.scalar.activation(tC3[:, :, :], C3_bf[:, :, sl], ACTF.Tanh)
            H3 = workB.tile([128, 4, 256], BF16, tag="H3", name=f"H3_{h}")
            with nc.allow_low_precision(LOWP):
                nc.vector.tensor_mul(H3[:, :, :], o3[:, :, :], tC3[:, :, :])
                tree16(hsum2B[:, :, g16], H3, f"hs2_{h}", g=16)
                nc.vector.tensor_scalar_mul(hsum2["h"][:, :, g16], hsum2B[:, :, g16], 1.0)
                nc.vector.tensor_scalar_mul(hsum2["d"][:, :, g16], hsum2B[:, :, g16], 1 / 16.0)
                nc.vector.tensor_sub(hsT2[:, :, :], hsum2B[:, :, g16], hsum2["h"][:, :, g16])
                nc.vector.tensor_scalar_mul(hsum2res[:, :, g16], hsT2[:, :, :], 16.0)

            # f-path to level 2 for this half: psum = 2048*pre_f2
            # wave1: wfh_h(256w) x C8_3(8*C3); wave2: wfh_l(16r) x C8_3d(C3/2)
            ps_f2 = psum.tile([128, 4, 512], F32, tag="ps")
            for m in range(4):
                ms = slice(m * 128, (m + 1) * 128)
                nc.tensor.matmul(
                    ps_f2[:, m, :256], fx21_nat[0:32, 0, ms], S_sb[0:32, sl],
                    start=True, stop=False, tile_position=(0, 0),
                )
                for wn, rr in (("wfh_h", C8_3), ("wfh_l", C8_3d), ("wfh_d", C3res)):
                    for kp in range(2):
                        nc.tensor.matmul(
                            ps_f2[:, m, :256], W[wn][:, 2 * kp : 2 * kp + 2, ms],
                            rr[:, 2 * kp : 2 * kp + 2, sl],
                            start=False, stop=(wn == "wfh_d" and kp == 1), perf_mode=DR,
                        )
            f2 = workB.tile([128, 4, 256], BF16, tag="U3", name=f"f2_{h}")
            evac(ps_f2, f2, ACTF.Sigmoid, 1 / 2048.0, n=256)
            fC2 = workB.tile([128, 4, 256], BF16, tag="B3", name=f"fC2_{h}")
            with nc.allow_low_precision(LOWP):
                nc.vector.tensor_mul(fC2[:, :, :], f2[:, :, :], C3_bf[:, :, sl])
                nc.vector.tensor_reduce(
                    fcsum2T[:, :, g16],
                    fC2[:, :, :].rearrange("p t (g w) -> p t g w", w=16),
                    axis=AX.X, op=OP.add,
                )

        # ---------------- level 2 (32 nodes, two 16-node halves) ----------------
        C2_bf = longp.tile([128, 4, 32], BF16, tag="C2_bf")
        C2_d = longp.tile([128, 4, 32], BF16, tag="C2_d")
        hsum1B = longp.tile([128, 4, 2], BF16, tag="hsum1B")
        hsum1 = {"h": longp.tile([128, 4, 2], F8, tag="hsum1h", name="hsum1h"),
                 "d": longp.tile([128, 4, 2], F8, tag="hsum1d", name="hsum1d")}
        hsum1res = longp.tile([128, 4, 2], F8, tag="hsum1res")
        hsT1 = longp.tile([128, 4, 1], BF16, tag="hsT1")
        fcsum1T = longp.tile([128, 4, 2], F32, tag="fcsum1T")

        def l2_half(h):
            g16 = slice(16 * h, 16 * h + 16)
            g1 = slice(h, h + 1)
            hp = (hsum2["h"], hsum2["d"], hsum2res)
            xtt = (xtail["h"], xtail["d"], xtail["l"])
            ps2 = dr_gate("i", xtt, n=16, hpart=hp, col0=16 * h)
            i2 = longp.tile([128, 4, 16], BF16, tag="s_i", name=f"i2_{h}")
            evac(ps2, i2, ACTF.Sigmoid, 1 / SPS, n=16)
            ps2 = dr_gate("o", xtt, n=16, hpart=hp, col0=16 * h)
            o2 = longp.tile([128, 4, 16], BF16, tag="s_o", name=f"o2_{h}")
            evac(ps2, o2, ACTF.Sigmoid, 1 / SPS, n=16)
            ps2 = dr_gate("u", xtt, n=16, hpart=hp, col0=16 * h)
            u2 = longp.tile([128, 4, 16], BF16, tag="s_u", name=f"u2_{h}")
            evac(ps2, u2, ACTF.Tanh, 1 / SPS, n=16)

            iu2 = longp.tile([128, 4, 16], BF16, tag="s_t", name=f"iu2_{h}")
            tC2 = longp.tile([128, 4, 16], BF16, tag="s_t2", name=f"tC2_{h}")
            H2 = longp.tile([128, 4, 16], BF16, tag="s_h", name=f"H2_{h}")
            with nc.allow_low_precision(LOWP):
                nc.vector.tensor_mul(iu2[:, :, :], i2[:, :, :], u2[:, :, :])
                nc.vector.tensor_add(C2_bf[:, :, g16], iu2[:, :, :], fcsum2T[:, :, g16])
                nc.vector.tensor_scalar_mul(C2_d[:, :, g16], C2_bf[:, :, g16], 1 / 16.0)
            nc.scalar.activation(tC2[:, :, :], C2_bf[:, :, g16], ACTF.Tanh)
            with nc.allow_low_precision(LOWP):
                nc.vector.tensor_mul(H2[:, :, :], o2[:, :, :], tC2[:, :, :])
                nc.vector.tensor_reduce(
                    hsum1B[:, :, g1],
                    H2[:, :, :].rearrange("p t (g w) -> p t g w", w=16),
                    axis=AX.X, op=OP.add,
                )
                nc.vector.tensor_scalar_mul(hsum1["h"][:, :, g1], hsum1B[:, :, g1], 1.0)
                nc.vector.tensor_scalar_mul(hsum1["d"][:, :, g1], hsum1B[:, :, g1], 1 / 16.0)
                nc.vector.tensor_sub(hsT1[:, :, :], hsum1B[:, :, g1], hsum1["h"][:, :, g1])
                nc.vector.tensor_scalar_mul(hsum1res[:, :, g1], hsT1[:, :, :], 16.0)

            # f-path to level 1 for this half: psum = 256*pre_f1
            # wave1: wfh_h(256w) x C2_bf (bf16); wave2: wfh_l(16r) x C2_d(C2/16)
            ps_f1 = psum.tile([128, 4, 512], F32, tag="ps")
            for m in range(4):
                ms = slice(m * 128, (m + 1) * 128)
                nc.tensor.matmul(
                    ps_f1[:, m, 0:16], fx21_nat[0:2, 1, ms], S_sb[0:2, g16],
                    start=True, stop=False, tile_position=(0, 0),
                )
                for wn, rr in (("wfh_h", C2_bf), ("wfh_l", C2_d)):
                    for k in range(4):
                        nc.tensor.matmul(
                            ps_f1[:, m, 0:16], W[wn][:, k, ms], rr[:, k, g16],
                            start=False, stop=(wn == "wfh_l" and k == 3),
                        )
            f1 = longp.tile([128, 4, 16], BF16, tag="s_f1", name=f"f1_{h}")
            evac(ps_f1, f1, ACTF.Sigmoid, 1 / 256.0, n=16)
            fC1 = longp.tile([128, 4, 16], F32, tag="s_fc1", name=f"fC1_{h}")
            with nc.allow_low_precision(LOWP):
                nc.vector.tensor_mul(fC1[:, :, :], f1[:, :, :], C2_bf[:, :, g16])
            nc.vector.tensor_reduce(
                fcsum1T[:, :, g1],
                fC1[:, :, :].rearrange("p t (g w) -> p t g w", w=16),
                axis=AX.X, op=OP.add,
            )

        Ct = longp.tile([128, 4, 512], BF16, tag="Ct")
        n_chunk = cfg.get("n_chunk", N_CHUNK)
        pipe = None  # (chunk index, C_bf, C8, C8d)
        for c in range(n_chunk):
            xt_c = stream_tiles.pop(c) if c in stream_tiles else load_chunk(c)
            if c + 1 < n_chunk and (c + 1) not in stream_tiles:
                load_chunk(c + 1)

            ps_i = dr_gate("i", xt_c)
            i_bf = workA.tile([128, 4, 512], BF16, tag="A")
            evac(ps_i, i_bf, ACTF.Sigmoid, 1 / SPS)

            ps_o = dr_gate("o", xt_c)
            o_bf = workB.tile([128, 4, 512], BF16, tag="B")
            evac(ps_o, o_bf, ACTF.Sigmoid, 1 / SPS)

            ps_u = dr_gate("u", xt_c)
            u_bf = workB.tile([128, 4, 512], BF16, tag="Ug")
            evac(ps_u, u_bf, ACTF.Tanh, 1 / SPS)

            C_bf = workA.tile([128, 4, 512], BF16, tag="C")
            C8 = workA.tile([128, 4, 512], F8, tag="C8")
            C8d = workA.tile([128, 4, 512], F8, tag="C8d")
            C8l = workA.tile([128, 4, 512], F8, tag="C8l")
            with nc.allow_low_precision(LOWP):
                nc.vector.tensor_mul(C_bf[:, :, :], i_bf[:, :, :], u_bf[:, :, :])
                nc.vector.tensor_scalar_mul(C8[:, :, :], C_bf[:, :, :], SX)
                nc.gpsimd.tensor_scalar_mul(C8d[:, :, :], C_bf[:, :, :], 1.0)
                # C8l = 16*(16*C - C8): the f-gate's C residual wave
                nc.vector.scalar_tensor_tensor(
                    Ct[:, :, :], C_bf[:, :, :], SX, C8[:, :, :],
                    op0=OP.mult, op1=OP.subtract,
                )
                nc.gpsimd.tensor_scalar_mul(C8l[:, :, :], Ct[:, :, :], 16.0)

            if c == 0:
                fx3_compute()
                if cfg.get("debug"):
                    nc.sync.dma_start(out=dbg["dbg_C0ch"][:, :, :], in_=C_bf[:, :, :])
            if c == 1:
                fx21_compute()
            if pipe is not None:
                leaf_fpath(*pipe)

            leaf_hpath(c, C_bf, o_bf)
            pipe = (c, C_bf, C8, C8d, C8l)

            if c == 8:
                l3_half(0)
            if c == 10:
                l2_half(0)

        leaf_fpath(*pipe)
        l3_half(1)
        l2_half(1)

        # ---------------- level 1 (2 nodes, transposed) ----------------
        x1v = (xtail["h"], xtail["d"], xtail["l"])
        hp1 = (hsum1["h"], hsum1["d"], hsum1res)
        ps1 = dr_gate("i", x1v, n=2, hpart=hp1, col0=32, hcol0=0)
        i1 = longp.tile([128, 4, 2], F32, tag="s_i1")
        evac(ps1, i1, ACTF.Sigmoid, 1 / SPS, n=2)
        ps1 = dr_gate("o", x1v, n=2, hpart=hp1, col0=32, hcol0=0)
        o1 = longp.tile([128, 4, 2], F32, tag="s_o1")
        evac(ps1, o1, ACTF.Sigmoid, 1 / SPS, n=2)
        ps1 = dr_gate("u", x1v, n=2, hpart=hp1, col0=32, hcol0=0)
        u1 = longp.tile([128, 4, 2], F32, tag="s_u1")
        evac(ps1, u1, ACTF.Tanh, 1 / SPS, n=2)

        iu1 = longp.tile([128, 4, 2], F32, tag="s_iu1")
        C1T = longp.tile([128, 4, 2], F32, tag="C1T")
        nc.vector.tensor_mul(iu1[:, :, :], i1[:, :, :], u1[:, :, :])
        nc.vector.tensor_add(C1T[:, :, :], iu1[:, :, :], fcsum1T[:, :, :])
        tC1 = longp.tile([128, 4, 2], F32, tag="s_tc1")
        nc.scalar.activation(tC1[:, :, :], C1T[:, :, :], ACTF.Tanh)
        H1T = longp.tile([128, 4, 2], F32, tag="s_h1")
        nc.vector.tensor_mul(H1T[:, :, :], o1[:, :, :], tC1[:, :, :])

        # transpose to natural layout and DMA out
        ps_t = psum.tile([128, 4, 512], F32, tag="ps")
        for t in range(4):
            nc.tensor.transpose(ps_t[0:2, 0, t * 128 : (t + 1) * 128], H1T[:, t, :], ident[:, :])
            nc.tensor.transpose(ps_t[0:2, 1, t * 128 : (t + 1) * 128], C1T[:, t, :], ident[:, :])
        out_sb = longp.tile([2, 2, 512], F32, tag="out_sb")
        nc.scalar.activation(out_sb[:, :, :], ps_t[0:2, 0:2, :], ACTF.Copy)
        nc.sync.dma_start(
            out=out_hc[:, :].rearrange("(hc n) d -> n hc d", n=2), in_=out_sb[:, :, :]
        )

        if cfg.get("debug"):
            for nm, tl_, cast in [
                ("dbg_hsum3", hsum3B, None), ("dbg_fcsum3", fcsum3T, None),
                ("dbg_C3", C3_bf, None), ("dbg_fx3", fx3_nat, F32),
                ("dbg_hsum2", hsum2B, None), ("dbg_fcsum2", fcsum2T, None),
                ("dbg_C2", C2_bf, None), ("dbg_hsum1", hsum1B, None),
                ("dbg_fcsum1", fcsum1T, None), ("dbg_C1", C1T, None),
            ]:
                src = tl_[:, :, :]
                if cast is not None:
                    src = src.bitcast(cast)
                nc.sync.dma_start(out=dbg[nm][:, :, :], in_=src)

    nc.finalize()
    return nc


def _np_sigmoid(v):
    return 1.0 / (1.0 + np.exp(-v))


def _host_prep(x, wi_w, wo_w, wu_w, wf_w, wi_b, wo_b, wu_b, wf_b):
    import ml_dtypes

    E4 = ml_dtypes.float8_e4m3

    def q8(a):
        return np.asarray(a, np.float32).astype(E4)

    xs = np.asarray(x, np.float32).T  # [512, 69905]
    xt8_full = q8(xs * SX)
    xt8d_full = q8(xs)
    xt8l_full = q8(16.0 * (xs * SX - xt8_full.astype(np.float32)))

    def wpair(w, s):
        wT = np.ascontiguousarray(np.asarray(w, np.float32).T) * s
        hi = q8(wT)
        lo = q8(16.0 * (wT - hi.astype(np.float32)))
        return hi, lo

    common = {}
    for nm, w, s in [
        ("wix", wi_w[:, :512], SWX), ("wih", wi_w[:, 512:], SWH),
        ("wox", wo_w[:, :512], SWX), ("woh", wo_w[:, 512:], SWH),
        ("wux", wu_w[:, :512], SWX), ("wuh", wu_w[:, 512:], SWH),
        ("wfx", wf_w[:, :512], SWX), ("wfh", wf_w[:, 512:], SWX),
    ]:
        hi, lo = wpair(w, s)
        common[nm + "_h"] = hi
        common[nm + "_l"] = lo
    common["wux_d"] = q8(np.ascontiguousarray(np.asarray(wu_w[:, :512], np.float32).T) * (SWX / 16.0))
    common["wix_d"] = q8(np.ascontiguousarray(np.asarray(wi_w[:, :512], np.float32).T) * (SWX / 16.0))
    common["wox_d"] = q8(np.ascontiguousarray(np.asarray(wo_w[:, :512], np.float32).T) * (SWX / 16.0))
    common["wuh_d"] = q8(np.ascontiguousarray(np.asarray(wu_w[:, 512:], np.float32).T) * (SWH / 16.0))
    common["wfh_d"] = q8(np.ascontiguousarray(np.asarray(wf_w[:, 512:], np.float32).T) * (SWX / 16.0))
    common["wfx_d"] = q8(np.ascontiguousarray(np.asarray(wf_w[:, :512], np.float32).T) * (SWX / 16.0))

    def bias8(b):
        bs = np.asarray(b, np.float32).reshape(4, 128) * SWX
        hi = bs.astype(E4)
        lo = (16.0 * (bs - hi.astype(np.float32))).astype(E4)
        out = np.zeros((1, 1024), E4)
        ov = out[0, :].reshape(4, 2, 128)
        ov[:, 0, :] = hi
        ov[:, 1, :] = lo
        return out

    ones8 = np.zeros((1, 1024), np.float32)
    ones8[0, 0:512] = SX
    ones8[0, 512:1024] = 1.0
    common.update({
        "bias8i": bias8(wi_b), "bias8o": bias8(wo_b), "bias8u": bias8(wu_b),
        "ones8": ones8.astype(E4),
        "b4096": np.ascontiguousarray(np.asarray(wf_b, np.float32)[None, :] * SPS),
        "S": (np.arange(512)[None, :] // 16 == (np.arange(128) % 32)[:, None]).astype(np.float32),
        "onescol": np.ones((1, 128), np.float32),
    })
    in_maps = []
    for c in range(NC_N):
        def cols(full):
            return np.ascontiguousarray(np.concatenate(
                [
                    full[:, OFFS[4] + 8192 * c : OFFS[4] + 8192 * (c + 1)],
                    full[:, OFFS[3] + 512 * c : OFFS[3] + 512 * (c + 1)],
                    full[:, OFFS[2] + 32 * c : OFFS[2] + 32 * (c + 1)],
                    full[:, OFFS[1] + 2 * c : OFFS[1] + 2 * (c + 1)],
                ],
                axis=1,
            ))
        in_maps.append({"xt8": cols(xt8_full), "xt8d": cols(xt8d_full),
                        "xt8l": cols(xt8l_full), **common})
    return in_maps


def _host_finish(x, H1_all, C1_all, wi_w, wi_b, wf_w, wf_b, wo_w, wo_b, wu_w, wu_b):
    """Level 0 (root): its 16 children are the level-1 nodes across cores."""
    f8 = np.float64
    x0 = np.asarray(x[0], f8)
    H1 = np.asarray(H1_all, f8)
    C1 = np.asarray(C1_all, f8)
    hsum0 = H1.sum(0)
    f0 = _np_sigmoid(
        x0 @ np.asarray(wf_w, f8)[:, :512].T + C1 @ np.asarray(wf_w, f8)[:, 512:].T + np.asarray(wf_b, f8)
    )
    fcsum0 = (f0 * C1).sum(0)
    xh0 = np.concatenate([x0, hsum0])
    i0 = _np_sigmoid(xh0 @ np.asarray(wi_w, f8).T + np.asarray(wi_b, f8))
    o0 = _np_sigmoid(xh0 @ np.asarray(wo_w, f8).T + np.asarray(wo_b, f8))
    u0 = np.tanh(xh0 @ np.asarray(wu_w, f8).T + np.asarray(wu_b, f8))
    C0 = i0 * u0 + fcsum0
    H0 = o0 * np.tanh(C0)
    return H0.astype(np.float32), C0.astype(np.float32)


def _run(in_maps, trace=False):
    from concourse.bass_utils import run_bass_kernel_spmd

    if "nc" not in _CACHE:
        _CACHE["nc"] = _build_nc()
    return run_bass_kernel_spmd(_CACHE["nc"], in_maps, list(range(NC_N)), trace=trace)


def kernel(x, wi_w, wi_b, wf_w, wf_b, wo_w, wo_b, wu_w, wu_b, _trace=False):
    x = np.asarray(x, np.float32)
    in_maps = _host_prep(x, wi_w, wo_w, wu_w, wf_w, wi_b, wo_b, wu_b, wf_b)
    res = _run(in_maps, trace=_trace)
    _CACHE["last_results"] = res
    H1_all = np.concatenate([res.results[c]["out_hc"][0:2] for c in range(NC_N)])
    C1_all = np.concatenate([res.results[c]["out_hc"][2:4] for c in range(NC_N)])
    H0, C0 = _host_finish(x, H1_all, C1_all, wi_w, wi_b, wf_w, wf_b, wo_w, wo_b, wu_w, wu_b)
    return H0, C0


# revision 69
# speedup vs baseline: 1.0909x; 1.0391x over previous
"""CS-TreeLSTM (BRANCH=16, DEPTH=4, IN=HID=512) on 8 Trainium2 NeuronCores.

Strategy (data-parallel over subtrees, per the sharding hint):
  - Each core owns 8192 leaves, 512 level-3, 32 level-2, 2 level-1 nodes.
  - Activations live transposed on-chip: [hid/in on partitions, nodes on free].
  - Gate matmuls run as fp8-e4m3 DoubleRow (256-deep contraction per
    instruction at 0.5 cycles/row).  Weight quantization error is systematic
    across nodes and amplifies ~8x per tree level through the 16-child sums,
    so every weight is sent as a hi+lo fp8 PAIR (wl = fp8(16*(s*w - wh)));
    each gate runs a hi wave and a residual wave, giving ~1.3e-3 effective
    weight precision (2nd order in fp8 eps) at half the f32r PE cost.
    Activation (x, C, hsum) quantization is iid across nodes and cancels in
    the sibling sums, so single fp8 is enough for i/o/f; the u gate (tanh
    slope 1, feeds C directly) also gets an x-residual wave.
  - Scales: x8=16x, x8d=x, xl8=16*(16x-x8); Wx hi=256w; Wh hi=4096w;
    every i/o/u psum = 4096*pre, one merged ACT evac with scale 1/4096.
  - Gate biases ride tiny DoubleRow aug matmuls (hi slice x ones=16, lo
    residual slice x ones=1), making evacs single merged instructions (ACT is
    the bottleneck engine).
  - Forget gates: psum = Wfh_hi x C8(16C) + Wfh_lo x C8d(C) + f32r
    indicator-matrix aug carrying s*(Wfx x_parent + b_f) (computed once per
    level from an exact psum, weight pair included).
  - Sibling sums: h-path via bf16 pairwise-add tree (2x DVE mode) ->
    bf16 -> fp8 hi/lo; fcsum via plain reduce with f32 output.
  - Level 3 runs as two 256-node halves, the first interleaved at leaf chunk
    8 so its PE/DVE work hides under leaf-phase ACT; level 2 likewise as two
    16-node halves (chunk 10 / tail).
  - Level 0 (root; children span all cores) is combined on the host from
    per-core H1/C1 outputs (the only cross-core communication, 8x8KB).
"""

import sys

sys.path.insert(0, "/opt/trn_rl_repo")

import numpy as np

BRANCH = 16
DEPTH = 4
IN = 512
HID = 512
NC_N = 8
SIZES = [BRANCH**d for d in range(DEPTH + 1)]  # [1,16,256,4096,65536]
OFFS = [0, 1, 17, 273, 4369, 69905]
XT_COLS = 8192 + 512 + 32 + 2  # 8738
C3_OFF = 8192
C2_OFF = 8192 + 512
N_CHUNK = 16

SX = 16.0     # fp8 x hi scale
SWX = 256.0   # fp8 x-part / fh weight hi scale
SWH = 4096.0  # fp8 h-part weight hi scale
SPS = 4096.0  # i/o/u psum scale

_CACHE = {}

WNAMES = ["wix", "wih", "wox", "woh", "wux", "wuh", "wfx", "wfh"]


def _build_nc(cfg=None):
    cfg = cfg or {}
    from concourse import bacc
    import concourse.mybir as mybir
    import concourse.tile as tile
    from concourse.masks import make_identity

    F32 = mybir.dt.float32
    F32R = mybir.dt.float32r
    BF16 = mybir.dt.float16  # fp16: 10-bit mantissa, same DVE speed as bf16
    F8 = mybir.dt.float8e4
    ACTF = mybir.ActivationFunctionType
    AX = mybir.AxisListType
    OP = mybir.AluOpType
    DR = mybir.MatmulPerfMode.DoubleRow

    nc = bacc.Bacc()

    xt8 = nc.declare_dram_parameter("xt8", [IN, XT_COLS], F8, isOutput=False)
    xt8d = nc.declare_dram_parameter("xt8d", [IN, XT_COLS], F8, isOutput=False)
    xt8l = nc.declare_dram_parameter("xt8l", [IN, XT_COLS], F8, isOutput=False)
    wps = {}
    for n in WNAMES:
        wps[n + "_h"] = nc.declare_dram_parameter(n + "_h", [IN, HID], F8, isOutput=False)
        wps[n + "_l"] = nc.declare_dram_parameter(n + "_l", [IN, HID], F8, isOutput=False)
    for n in ("wux_d", "wix_d", "wox_d", "wuh_d", "wfh_d", "wfx_d"):
        wps[n] = nc.declare_dram_parameter(n, [IN, HID], F8, isOutput=False)
    bias8p = {g: nc.declare_dram_parameter("bias8" + g, [1, 1024], F8, isOutput=False) for g in "iou"}
    ones8p = nc.declare_dram_parameter("ones8", [1, 1024], F8, isOutput=False)
    b4096p = nc.declare_dram_parameter("b4096", [1, HID], F32, isOutput=False)
    S_p = nc.declare_dram_parameter("S", [128, 512], F32, isOutput=False)
    S8p = nc.declare_dram_parameter("S8", [128, 1024], F8, isOutput=False)
    onescolp = nc.declare_dram_parameter("onescol", [1, 128], F32, isOutput=False)
    out_hc = nc.declare_dram_parameter("out_hc", [4, HID], F32, isOutput=True)
    dbg = {}
    if cfg.get("debug"):
        for nm, shp, dt_ in [
            ("dbg_hsum3", [128, 4, 512], BF16), ("dbg_fcsum3", [128, 4, 512], F32),
            ("dbg_C3", [128, 4, 512], BF16), ("dbg_fx3", [128, 4, 512], F32),
            ("dbg_hsum2", [128, 4, 32], BF16), ("dbg_fcsum2", [128, 4, 32], F32),
            ("dbg_C2", [128, 4, 32], BF16), ("dbg_hsum1", [128, 4, 2], BF16),
            ("dbg_fcsum1", [128, 4, 2], F32), ("dbg_C1", [128, 4, 2], F32),
            ("dbg_C0ch", [128, 4, 512], BF16),
        ]:
            dbg[nm] = nc.declare_dram_parameter(nm, shp, dt_, isOutput=True)

    def t_view(h):  # DRAM [512, n] -> [128 part, 4 ktile, n] view
        return h[:, :].rearrange("(t p) n -> p t n", p=128)

    from contextlib import ExitStack

    with tile.TileContext(nc) as tc, ExitStack() as ctx:
        consts = ctx.enter_context(tc.tile_pool(name="consts", bufs=1))
        stream = ctx.enter_context(tc.tile_pool(name="stream", bufs=cfg.get("stream", 2)))
        workA = ctx.enter_context(tc.tile_pool(name="workA", bufs=2))
        workB = ctx.enter_context(tc.tile_pool(name="workB", bufs=2))
        longp = ctx.enter_context(tc.tile_pool(name="longp", bufs=1))
        psum = ctx.enter_context(tc.tile_pool(name="psum", bufs=2, space="PSUM"))

        LOWP = "paired-fp8/bf16 pipeline, tolerance 2e-2"

        # ---------------- constants / weights ----------------
        bias8 = {}
        for g in "iou":
            bias8[g] = consts.tile([1, 4, 2, 128], F8, tag="bias8" + g, name="bias8" + g)
            nc.sync.dma_start(
                out=bias8[g][:, :, :, :],
                in_=bias8p[g][:, :].rearrange("a (m two f) -> a m two f", two=2, f=128),
            )
        ones8 = consts.tile([1, 2, 512], F8, tag="ones8")
        nc.sync.dma_start(out=ones8[:, :, :], in_=ones8p[:, :].rearrange("a (two f) -> a two f", two=2))

        stream_tiles = {}

        def load_chunk(c):
            th = stream.tile([128, 4, 512], F8, tag="xt_c", name=f"xt_c{c}")
            nc.sync.dma_start(out=th[:, :, :], in_=t_view(xt8)[:, :, c * 512 : (c + 1) * 512])
            td = stream.tile([128, 4, 512], F8, tag="xt_d", name=f"xt_d{c}")
            nc.sync.dma_start(out=td[:, :, :], in_=t_view(xt8d)[:, :, c * 512 : (c + 1) * 512])
            tl = stream.tile([128, 4, 512], F8, tag="xt_l", name=f"xt_l{c}")
            nc.sync.dma_start(out=tl[:, :, :], in_=t_view(xt8l)[:, :, c * 512 : (c + 1) * 512])
            stream_tiles[c] = (th, td, tl)
            return stream_tiles[c]

        W = {}

        def load_w(n):
            W[n] = consts.tile([128, 4, HID], F8, tag="w_" + n, name="w_" + n)
            nc.sync.dma_start(out=W[n][:, :, :], in_=t_view(wps[n]))

        load_chunk(0)
        load_w("wix_h")
        load_w("wix_l")
        load_w("wox_h")
        load_w("wox_l")
        load_w("wux_h")
        load_w("wux_l")
        load_w("wux_d")
        load_w("wix_d")
        load_w("wox_d")
        load_w("wuh_d")
        load_chunk(1)
        xt3 = {}
        for sfx, src in (("h", xt8), ("d", xt8d), ("l", xt8l)):
            xt3[sfx] = consts.tile([128, 4, 512], F8, tag="xt3" + sfx, name="xt3" + sfx)
            nc.sync.dma_start(out=xt3[sfx][:, :, :], in_=t_view(src)[:, :, C3_OFF : C3_OFF + 512])
        load_w("wfx_h")
        load_w("wfx_l")
        load_w("wfx_d")
        onescol = consts.tile([1, 128], F32R, tag="onescol")
        nc.sync.dma_start(out=onescol[:, :], in_=onescolp[:, :].bitcast(F32R))
        b4096 = consts.tile([1, 512], F32R, tag="b4096")
        nc.sync.dma_start(out=b4096[:, :], in_=b4096p[:, :].bitcast(F32R))
        S_sb = consts.tile([128, 512], F32R, tag="S")
        nc.sync.dma_start(out=S_sb[:, :], in_=S_p[:, :].bitcast(F32R))
        S8_sb = consts.tile([128, 2, 512], F8, tag="S8")
        nc.sync.dma_start(out=S8_sb[:, :, :], in_=S8p[:, :].rearrange("p (two n) -> p two n", two=2))
        load_w("wfh_h")
        load_w("wfh_l")
        load_w("wfh_d")
        load_w("wih_h")
        load_w("wih_l")
        load_w("woh_h")
        load_w("woh_l")
        load_w("wuh_h")
        load_w("wuh_l")
        # tail x: [128,4,34] views for moving, clean-stride [128,4,32] for
        # fx stationaries (dual-fp8 Ldweights needs plane stride % 32 == 0)
        xtail = {}
        for sfx, src in (("h", xt8), ("d", xt8d), ("l", xt8l)):
            xtail[sfx] = consts.tile([128, 4, 34], F8, tag="xtail" + sfx, name="xtail" + sfx)
            nc.sync.dma_start(out=xtail[sfx][:, :, :], in_=t_view(src)[:, :, C2_OFF : C2_OFF + 34])
        xt2s = {}
        xt1s = {}
        for sfx, src in (("h", xt8), ("d", xt8d), ("l", xt8l)):
            xt2s[sfx] = consts.tile([128, 4, 32], F8, tag="xt2s" + sfx, name="xt2s" + sfx)
            nc.sync.dma_start(out=xt2s[sfx][:, :, :], in_=t_view(src)[:, :, C2_OFF : C2_OFF + 32])
            xt1s[sfx] = consts.tile([128, 4, 32], F8, tag="xt1s" + sfx, name="xt1s" + sfx)
            nc.sync.dma_start(out=xt1s[sfx][:, :, 0:2], in_=t_view(src)[:, :, C2_OFF + 32 : C2_OFF + 34])
        ident = consts.tile([128, 128], F32, tag="ident")
        make_identity(nc, ident[:, :])

        # persistent accumulators
        hsum3B = longp.tile([128, 4, 512], BF16, tag="hsum3B")
        hsum3 = {"h": longp.tile([128, 4, 512], F8, tag="hsum3h", name="hsum3h"),
                 "d": longp.tile([128, 4, 512], F8, tag="hsum3d", name="hsum3d")}
        hsum3res = longp.tile([128, 4, 512], F8, tag="hsum3res")
        hsT3 = longp.tile([128, 4, 32], BF16, tag="hsT3")
        fcsum3T = longp.tile([128, 4, 512], F32, tag="fcsum3T")
        fx3_nat = longp.tile([128, 4, 512], F32R, tag="fx3_nat")  # 4096*(fx3+bf)
        fx38 = longp.tile([128, 4, 2, 512], F8, tag="fx38")  # hi/lo pair at scale 64/16x
        # [0:32, 0, :] = 2048*(fx2+bf); [0:2, 1, :] = 256*(fx1+bf)
        fx21_nat = longp.tile([128, 2, 512], F32R, tag="fx21_nat")

        def dr_gate(g, rhs, n=512, hpart=None, col0=0, hcol0=None):
            """i/o/u gate psum: 4096*(W x (+ W h) + b) via paired-fp8 DR waves.
            rhs = (hi, d, l) x tiles; hpart = (hi8, d8) hsum tiles.
            u gets a 3rd x wave (x residual, tanh slope 1 feeds C directly)."""
            if hcol0 is None:
                hcol0 = col0
            ps = psum.tile([128, 4, 512], F32, tag="ps")
            for m in range(4):
                nc.tensor.matmul(
                    ps[:, m, :n], bias8[g][0:1, m, :, :], ones8[:, :, :n],
                    start=True, stop=False, perf_mode=DR,
                )
            if g == "o":
                waves = [("wox_h", rhs[0], col0), ("wox_l", rhs[1], col0)]
            else:
                waves = [("w" + g + "x_h", rhs[0], col0), ("w" + g + "x_l", rhs[1], col0),
                         ("w" + g + "x_d", rhs[2], col0)]
            if hpart is not None:
                waves.append(("w" + g + "h_h", hpart[0], hcol0))
                waves.append(("w" + g + "h_l", hpart[1], hcol0))
                if g == "u":
                    waves.append(("wuh_d", hpart[2], hcol0))
            for wi, (wn, rr, c0) in enumerate(waves):
                last_wave = wi == len(waves) - 1
                for m in range(4):
                    ms = slice(m * 128, (m + 1) * 128)
                    for kp in range(2):
                        nc.tensor.matmul(
                            ps[:, m, :n], W[wn][:, 2 * kp : 2 * kp + 2, ms],
                            rr[:, 2 * kp : 2 * kp + 2, c0 : c0 + n],
                            start=False, stop=(last_wave and kp == 1), perf_mode=DR,
                        )
            return ps

        def evac(ps, out_sb, act, scale, n=512):
            nc.scalar.activation(out_sb[:, :, :n], ps[:, :, :n], act, scale=scale)

        def f_gate_dr(C8, C8d, C8l, fx_sb, fx_rows, fx_col, n=512):
            """forget gate psum: f32r indicator aug (fx+bias) + paired fp8 DR on C."""
            ps = psum.tile([128, 4, 512], F32, tag="ps")
            b0, b1 = fx_rows
            for m in range(4):
                ms = slice(m * 128, (m + 1) * 128)
                nc.tensor.matmul(
                    ps[:, m, :n], fx_sb[b0:b1, fx_col, :, ms], S8_sb[b0:b1, :, :n],
                    start=True, stop=False, tile_position=(b0, 0), perf_mode=DR,
                )
                for wn, rr in (("wfh_h", C8), ("wfh_l", C8d), ("wfh_d", C8l)):
                    for kp in range(2):
                        nc.tensor.matmul(
                            ps[:, m, :n], W[wn][:, 2 * kp : 2 * kp + 2, ms],
                            rr[:, 2 * kp : 2 * kp + 2, :n],
                            start=False, stop=(wn == "wfh_d" and kp == 1), perf_mode=DR,
                        )
            return ps

        def fx3_compute():
            # fx3_nat = 4096*(x3 @ WfxT + bf)  (natural layout, nodes on partitions)
            psx = psum.tile([128, 4, 512], F32, tag="ps")
            for pb in range(4):
                pbs = slice(pb * 128, (pb + 1) * 128)
                nc.tensor.matmul(psx[:, pb, :], onescol[:, :], b4096[:, :], start=True, stop=False)
                for sfx, wn in (("h", "wfx_h"), ("d", "wfx_l"), ("l", "wfx_d")):
                    for kp in range(2):
                        nc.tensor.matmul(
                            psx[:, pb, :], xt3[sfx][:, 2 * kp : 2 * kp + 2, pbs],
                            W[wn][:, 2 * kp : 2 * kp + 2, :],
                            start=False, stop=(sfx == "l" and kp == 1), perf_mode=DR,
                        )
            nc.scalar.activation(fx3_nat[:, :, :], psx[:, :, :], ACTF.Copy)
            nc.scalar.activation(fx38[:, :, 0, :], psx[:, :, :], ACTF.Copy, scale=1 / 64.0)
            fxT = workB.tile([128, 4, 512], BF16, tag="H", name="fxT")
            with nc.allow_low_precision(LOWP):
                nc.vector.scalar_tensor_tensor(
                    fxT[:, :, :], psx[:, :, :], 1 / 64.0, fx38[:, :, 0, :],
                    op0=OP.mult, op1=OP.subtract,
                )
                nc.vector.tensor_scalar_mul(fx38[:, :, 1, :], fxT[:, :, :], 16.0)

        def fx21_compute():
            ps = psum.tile([128, 4, 512], F32, tag="ps")
            nc.tensor.matmul(ps[0:32, 0, :], onescol[:, 0:32], b4096[:, :], start=True, stop=False)
            for sfx, wn in (("h", "wfx_h"), ("d", "wfx_l"), ("l", "wfx_d")):
                for kp in range(2):
                    nc.tensor.matmul(
                        ps[0:32, 0, :], xt2s[sfx][:, 2 * kp : 2 * kp + 2, :],
                        W[wn][:, 2 * kp : 2 * kp + 2, :],
                        start=False, stop=(sfx == "l" and kp == 1), perf_mode=DR,
                    )
            nc.tensor.matmul(ps[0:2, 1, :], onescol[:, 0:2], b4096[:, :], start=True, stop=False)
            for sfx, wn in (("h", "wfx_h"), ("d", "wfx_l"), ("l", "wfx_d")):
                for kp in range(2):
                    nc.tensor.matmul(
                        ps[0:2, 1, :], xt1s[sfx][:, 2 * kp : 2 * kp + 2, 0:2],
                        W[wn][:, 2 * kp : 2 * kp + 2, :],
                        start=False, stop=(sfx == "l" and kp == 1), perf_mode=DR,
                    )
            nc.scalar.activation(fx21_nat[0:32, 0, :], ps[0:32, 0, :], ACTF.Copy, scale=1 / 2.0)
            nc.scalar.activation(fx21_nat[0:2, 1, :], ps[0:2, 1, :], ACTF.Copy, scale=1 / 16.0)

        # ---------------- leaf phase ----------------
        def tree16(out_ap, src, pfx, g=32):
            # grouped sum of 16 along the free dim via bf16 pairwise adds (2x DVE mode)
            v = src[:, :, :].rearrange("p t (g w) -> p t g w", w=16)
            a1 = workB.tile([128, 4, g, 8], BF16, tag=f"tr1_{g}", name=pfx + "a1")
            a2 = workB.tile([128, 4, g, 4], BF16, tag=f"tr2_{g}", name=pfx + "a2")
            a3 = workB.tile([128, 4, g, 2], BF16, tag=f"tr3_{g}", name=pfx + "a3")
            nc.vector.tensor_add(a1[:, :, :, :], v[:, :, :, 0:8], v[:, :, :, 8:16])
            nc.vector.tensor_add(a2[:, :, :, :], a1[:, :, :, 0:4], a1[:, :, :, 4:8])
            nc.vector.tensor_add(a3[:, :, :, :], a2[:, :, :, 0:2], a2[:, :, :, 2:4])
            nc.vector.tensor_add(out_ap, a3[:, :, :, 0], a3[:, :, :, 1])

        def leaf_fpath(c, C_prev, C8_prev, C8d_prev, C8l_prev):
            b, pt = 32 * (c % 4), c // 4
            ps_f = f_gate_dr(C8_prev, C8d_prev, C8l_prev, fx38, (b, b + 32), pt)
            f_bf = workB.tile([128, 4, 512], BF16, tag="Ug")
            evac(ps_f, f_bf, ACTF.Sigmoid, 1 / SPS)
            fC = workB.tile([128, 4, 512], BF16, tag="H")
            with nc.allow_low_precision(LOWP):
                nc.vector.tensor_mul(fC[:, :, :], f_bf[:, :, :], C_prev[:, :, :])
                nc.vector.tensor_reduce(
                    fcsum3T[:, :, 32 * c : 32 * c + 32],
                    fC[:, :, :].rearrange("p t (g w) -> p t g w", w=16),
                    axis=AX.X, op=OP.add,
                )

        def leaf_hpath(c, C_bf, o_bf):
            tC = workA.tile([128, 4, 512], BF16, tag="A")
            nc.scalar.activation(tC[:, :, :], C_bf[:, :, :], ACTF.Tanh)
            H = workB.tile([128, 4, 512], BF16, tag="H")
            cols = slice(32 * c, 32 * c + 32)
            with nc.allow_low_precision(LOWP):
                nc.vector.tensor_mul(H[:, :, :], o_bf[:, :, :], tC[:, :, :])
                tree16(hsum3B[:, :, cols], H, f"hs{c}")
                nc.vector.tensor_scalar_mul(hsum3["h"][:, :, cols], hsum3B[:, :, cols], 1.0)
                nc.vector.tensor_scalar_mul(hsum3["d"][:, :, cols], hsum3B[:, :, cols], 1 / 16.0)
                nc.vector.tensor_sub(hsT3[:, :, :], hsum3B[:, :, cols], hsum3["h"][:, :, cols])
                nc.vector.tensor_scalar_mul(hsum3res[:, :, cols], hsT3[:, :, :], 16.0)

        # ---------------- level 3 (512 nodes, two 256-node halves) ----------------
        C3_bf = longp.tile([128, 4, 512], BF16, tag="C3_bf")
        C8_3 = longp.tile([128, 4, 512], F8, tag="C8_3")
        C8_3d = longp.tile([128, 4, 512], F8, tag="C8_3d")
        C3res = longp.tile([128, 4, 512], F8, tag="C3res")
        hsum2B = longp.tile([128, 4, 32], BF16, tag="hsum2B")
        hsum2 = {"h": longp.tile([128, 4, 32], F8, tag="hsum2h", name="hsum2h"),
                 "d": longp.tile([128, 4, 32], F8, tag="hsum2d", name="hsum2d")}
        hsum2res = longp.tile([128, 4, 32], F8, tag="hsum2res")
        hsT2 = longp.tile([128, 4, 16], BF16, tag="hsT2")
        fcsum2T = longp.tile([128, 4, 32], F32, tag="fcsum2T")

        def l3_half(h):
            sl = slice(256 * h, 256 * h + 256)
            g16 = slice(16 * h, 16 * h + 16)
            hp = (hsum3["h"], hsum3["d"], hsum3res)
            x3t = (xt3["h"], xt3["d"], xt3["l"])
            ps3 = dr_gate("i", x3t, n=256, hpart=hp, col0=256 * h)
            i3 = workA.tile([128, 4, 256], BF16, tag="A3", name=f"i3_{h}")
            evac(ps3, i3, ACTF.Sigmoid, 1 / SPS, n=256)
            ps3 = dr_gate("o", x3t, n=256, hpart=hp, col0=256 * h)
            o3 = workB.tile([128, 4, 256], BF16, tag="B3", name=f"o3_{h}")
            evac(ps3, o3, ACTF.Sigmoid, 1 / SPS, n=256)
            ps3 = dr_gate("u", x3t, n=256, hpart=hp, col0=256 * h)
            u3 = workB.tile([128, 4, 256], BF16, tag="U3", name=f"u3_{h}")
            evac(ps3, u3, ACTF.Tanh, 1 / SPS, n=256)

            iu3 = workB.tile([128, 4, 256], BF16, tag="H3", name=f"iu3_{h}")
            C3t = workB.tile([128, 4, 256], BF16, tag="B3", name=f"C3t_{h}")
            with nc.allow_low_precision(LOWP):
                nc.vector.tensor_mul(iu3[:, :, :], i3[:, :, :], u3[:, :, :])
                nc.vector.tensor_add(C3_bf[:, :, sl], iu3[:, :, :], fcsum3T[:, :, sl])
                nc.vector.tensor_scalar_mul(C8_3[:, :, sl], C3_bf[:, :, sl], 8.0)
                nc.gpsimd.tensor_scalar_mul(C8_3d[:, :, sl], C3_bf[:, :, sl], 0.5)
                # C3res = 16*(8*C3 - C8_3): f2's C residual wave
                nc.vector.scalar_tensor_tensor(
                    C3t[:, :, :], C3_bf[:, :, sl], 8.0, C8_3[:, :, sl],
                    op0=OP.mult, op1=OP.subtract,
                )
                nc.gpsimd.tensor_scalar_mul(C3res[:, :, sl], C3t[:, :, :], 16.0)
            tC3 = workA.tile([128, 4, 256], BF16, tag="A3", name=f"tC3_{h}")
            nc.scalar.activation(tC3[:, :, :], C3_bf[:, :, sl], ACTF.Tanh)
            H3 = workB.tile([128, 4, 256], BF16, tag="H3", name=f"H3_{h}")
            with nc.allow_low_precision(LOWP):
                nc.vector.tensor_mul(H3[:, :, :], o3[:, :, :], tC3[:, :, :])
                tree16(hsum2B[:, :, g16], H3, f"hs2_{h}", g=16)
                nc.vector.tensor_scalar_mul(hsum2["h"][:, :, g16], hsum2B[:, :, g16], 1.0)
                nc.vector.tensor_scalar_mul(hsum2["d"][:, :, g16], hsum2B[:, :, g16], 1 / 16.0)
                nc.vector.tensor_sub(hsT2[:, :, :], hsum2B[:, :, g16], hsum2["h"][:, :, g16])
                nc.vector.tensor_scalar_mul(hsum2res[:, :, g16], hsT2[:, :, :], 16.0)

            # f-path to level 2 for this half: psum = 2048*pre_f2
            # wave1: wfh_h(256w) x C8_3(8*C3); wave2: wfh_l(16r) x C8_3d(C3/2)
            ps_f2 = psum.tile([128, 4, 512], F32, tag="ps")
            for m in range(4):
                ms = slice(m * 128, (m + 1) * 128)
                nc.tensor.matmul(
                    ps_f2[:, m, :256], fx21_nat[0:32, 0, ms], S_sb[0:32, sl],
                    start=True, stop=False, tile_position=(0, 0),
                )
                for wn, rr in (("wfh_h", C8_3), ("wfh_l", C8_3d), ("wfh_d", C3res)):
                    for kp in range(2):
                        nc.tensor.matmul(
                            ps_f2[:, m, :256], W[wn][:, 2 * kp : 2 * kp + 2, ms],
                            rr[:, 2 * kp : 2 * kp + 2, sl],
                            start=False, stop=(wn == "wfh_d" and kp == 1), perf_mode=DR,
                        )
            f2 = workB.tile([128, 4, 256], BF16, tag="U3", name=f"f2_{h}")
            evac(ps_f2, f2, ACTF.Sigmoid, 1 / 2048.0, n=256)
            fC2 = workB.tile([128, 4, 256], BF16, tag="B3", name=f"fC2_{h}")
            with nc.allow_low_precision(LOWP):
                nc.vector.tensor_mul(fC2[:, :, :], f2[:, :, :], C3_bf[:, :, sl])
                nc.vector.tensor_reduce(
                    fcsum2T[:, :, g16],
                    fC2[:, :, :].rearrange("p t (g w) -> p t g w", w=16),
                    axis=AX.X, op=OP.add,
                )

        # ---------------- level 2 (32 nodes, two 16-node halves) ----------------
        C2_bf = longp.tile([128, 4, 32], BF16, tag="C2_bf")
        C2_d = longp.tile([128, 4, 32], BF16, tag="C2_d")
        hsum1B = longp.tile([128, 4, 2], BF16, tag="hsum1B")
        hsum1 = {"h": longp.tile([128, 4, 2], F8, tag="hsum1h", name="hsum1h"),
                 "d": longp.tile([128, 4, 2], F8, tag="hsum1d", name="hsum1d")}
        hsum1res = longp.tile([128, 4, 2], F8, tag="hsum1res")
        hsT1 = longp.tile([128, 4, 1], BF16, tag="hsT1")
        fcsum1T = longp.tile([128, 4, 2], F32, tag="fcsum1T")

        def l2_half(h):
            g16 = slice(16 * h, 16 * h + 16)
            g1 = slice(h, h + 1)
            hp = (hsum2["h"], hsum2["d"], hsum2res)
            xtt = (xtail["h"], xtail["d"], xtail["l"])
            ps2 = dr_gate("i", xtt, n=16, hpart=hp, col0=16 * h)
            i2 = longp.tile([128, 4, 16], BF16, tag="s_i", name=f"i2_{h}")
            evac(ps2, i2, ACTF.Sigmoid, 1 / SPS, n=16)
            ps2 = dr_gate("o", xtt, n=16, hpart=hp, col0=16 * h)
            o2 = longp.tile([128, 4, 16], BF16, tag="s_o", name=f"o2_{h}")
            evac(ps2, o2, ACTF.Sigmoid, 1 / SPS, n=16)
            ps2 = dr_gate("u", xtt, n=16, hpart=hp, col0=16 * h)
            u2 = longp.tile([128, 4, 16], BF16, tag="s_u", name=f"u2_{h}")
            evac(ps2, u2, ACTF.Tanh, 1 / SPS, n=16)

            iu2 = longp.tile([128, 4, 16], BF16, tag="s_t", name=f"iu2_{h}")
            tC2 = longp.tile([128, 4, 16], BF16, tag="s_t2", name=f"tC2_{h}")
            H2 = longp.tile([128, 4, 16], BF16, tag="s_h", name=f"H2_{h}")
            with nc.allow_low_precision(LOWP):
                nc.vector.tensor_mul(iu2[:, :, :], i2[:, :, :], u2[:, :, :])
                nc.vector.tensor_add(C2_bf[:, :, g16], iu2[:, :, :], fcsum2T[:, :, g16])
                nc.vector.tensor_scalar_mul(C2_d[:, :, g16], C2_bf[:, :, g16], 1 / 16.0)
            nc.scalar.activation(tC2[:, :, :], C2_bf[:, :, g16], ACTF.Tanh)
            with nc.allow_low_precision(LOWP):
                nc.vector.tensor_mul(H2[:, :, :], o2[:, :, :], tC2[:, :, :])
                nc.vector.tensor_reduce(
                    hsum1B[:, :, g1],
                    H2[:, :, :].rearrange("p t (g w) -> p t g w", w=16),
                    axis=AX.X, op=OP.add,
                )
                nc.vector.tensor_scalar_mul(hsum1["h"][:, :, g1], hsum1B[:, :, g1], 1.0)
                nc.vector.tensor_scalar_mul(hsum1["d"][:, :, g1], hsum1B[:, :, g1], 1 / 16.0)
                nc.vector.tensor_sub(hsT1[:, :, :], hsum1B[:, :, g1], hsum1["h"][:, :, g1])
                nc.vector.tensor_scalar_mul(hsum1res[:, :, g1], hsT1[:, :, :], 16.0)

            # f-path to level 1 for this half: psum = 256*pre_f1
            # wave1: wfh_h(256w) x C2_bf (bf16); wave2: wfh_l(16r) x C2_d(C2/16)
            ps_f1 = psum.tile([128, 4, 512], F32, tag="ps")
            for m in range(4):
                ms = slice(m * 128, (m + 1) * 128)
                nc.tensor.matmul(
                    ps_f1[:, m, 0:16], fx21_nat[0:2, 1, ms], S_sb[0:2, g16],
                    start=True, stop=False, tile_position=(0, 0),
                )
                for wn, rr in (("wfh_h", C2_bf), ("wfh_l", C2_d)):
                    for k in range(4):
                        nc.tensor.matmul(
                            ps_f1[:, m, 0:16], W[wn][:, k, ms], rr[:, k, g16],
                            start=False, stop=(wn == "wfh_l" and k == 3),
                        )
            f1 = longp.tile([128, 4, 16], BF16, tag="s_f1", name=f"f1_{h}")
            evac(ps_f1, f1, ACTF.Sigmoid, 1 / 256.0, n=16)
            fC1 = longp.tile([128, 4, 16], F32, tag="s_fc1", name=f"fC1_{h}")
            with nc.allow_low_precision(LOWP):
                nc.vector.tensor_mul(fC1[:, :, :], f1[:, :, :], C2_bf[:, :, g16])
            nc.vector.tensor_reduce(
                fcsum1T[:, :, g1],
                fC1[:, :, :].rearrange("p t (g w) -> p t g w", w=16),
                axis=AX.X, op=OP.add,
            )

        Ct = longp.tile([128, 4, 512], BF16, tag="Ct")
        n_chunk = cfg.get("n_chunk", N_CHUNK)
        pipe = None  # (chunk index, C_bf, C8, C8d)
        for c in range(n_chunk):
            xt_c = stream_tiles.pop(c) if c in stream_tiles else load_chunk(c)
            if c + 1 < n_chunk and (c + 1) not in stream_tiles:
                load_chunk(c + 1)

            ps_i = dr_gate("i", xt_c)
            i_bf = workA.tile([128, 4, 512], BF16, tag="A")
            evac(ps_i, i_bf, ACTF.Sigmoid, 1 / SPS)

            ps_o = dr_gate("o", xt_c)
            o_bf = workB.tile([128, 4, 512], BF16, tag="B")
            evac(ps_o, o_bf, ACTF.Sigmoid, 1 / SPS)

            ps_u = dr_gate("u", xt_c)
            u_bf = workB.tile([128, 4, 512], BF16, tag="Ug")
            evac(ps_u, u_bf, ACTF.Tanh, 1 / SPS)

            C_bf = workA.tile([128, 4, 512], BF16, tag="C")
            C8 = workA.tile([128, 4, 512], F8, tag="C8")
            C8d = workA.tile([128, 4, 512], F8, tag="C8d")
            C8l = workA.tile([128, 4, 512], F8, tag="C8l")
            with nc.allow_low_precision(LOWP):
                nc.vector.tensor_mul(C_bf[:, :, :], i_bf[:, :, :], u_bf[:, :, :])
                nc.vector.tensor_scalar_mul(C8[:, :, :], C_bf[:, :, :], SX)
                nc.gpsimd.tensor_scalar_mul(C8d[:, :, :], C_bf[:, :, :], 1.0)
                # C8l = 16*(16*C - C8): the f-gate's C residual wave
                nc.vector.scalar_tensor_tensor(
                    Ct[:, :, :], C_bf[:, :, :], SX, C8[:, :, :],
                    op0=OP.mult, op1=OP.subtract,
                )
                nc.gpsimd.tensor_scalar_mul(C8l[:, :, :], Ct[:, :, :], 16.0)

            if c == 0:
                fx3_compute()
                if cfg.get("debug"):
                    nc.sync.dma_start(out=dbg["dbg_C0ch"][:, :, :], in_=C_bf[:, :, :])
            if c == 1:
                fx21_compute()
            if pipe is not None:
                leaf_fpath(*pipe)

            leaf_hpath(c, C_bf, o_bf)
            pipe = (c, C_bf, C8, C8d, C8l)

            if c == 8:
                l3_half(0)
            if c == 10:
                l2_half(0)

        leaf_fpath(*pipe)
        l3_half(1)
        l2_half(1)

        # ---------------- level 1 (2 nodes, transposed) ----------------
        x1v = (xtail["h"], xtail["d"], xtail["l"])
        hp1 = (hsum1["h"], hsum1["d"], hsum1res)
        ps1 = dr_gate("i", x1v, n=2, hpart=hp1, col0=32, hcol0=0)
        i1 = longp.tile([128, 4, 2], F32, tag="s_i1")
        evac(ps1, i1, ACTF.Sigmoid, 1 / SPS, n=2)
        ps1 = dr_gate("o", x1v, n=2, hpart=hp1, col0=32, hcol0=0)
        o1 = longp.tile([128, 4, 2], F32, tag="s_o1")
        evac(ps1, o1, ACTF.Sigmoid, 1 / SPS, n=2)
        ps1 = dr_gate("u", x1v, n=2, hpart=hp1, col0=32, hcol0=0)
        u1 = longp.tile([128, 4, 2], F32, tag="s_u1")
        evac(ps1, u1, ACTF.Tanh, 1 / SPS, n=2)

        iu1 = longp.tile([128, 4, 2], F32, tag="s_iu1")
        C1T = longp.tile([128, 4, 2], F32, tag="C1T")
        nc.vector.tensor_mul(iu1[:, :, :], i1[:, :, :], u1[:, :, :])
        nc.vector.tensor_add(C1T[:, :, :], iu1[:, :, :], fcsum1T[:, :, :])
        tC1 = longp.tile([128, 4, 2], F32, tag="s_tc1")
        nc.scalar.activation(tC1[:, :, :], C1T[:, :, :], ACTF.Tanh)
        H1T = longp.tile([128, 4, 2], F32, tag="s_h1")
        nc.vector.tensor_mul(H1T[:, :, :], o1[:, :, :], tC1[:, :, :])

        # transpose to natural layout and DMA out
        ps_t = psum.tile([128, 4, 512], F32, tag="ps")
        for t in range(4):
            nc.tensor.transpose(ps_t[0:2, 0, t * 128 : (t + 1) * 128], H1T[:, t, :], ident[:, :])
            nc.tensor.transpose(ps_t[0:2, 1, t * 128 : (t + 1) * 128], C1T[:, t, :], ident[:, :])
        out_sb = longp.tile([2, 2, 512], F32, tag="out_sb")
        nc.scalar.activation(out_sb[:, :, :], ps_t[0:2, 0:2, :], ACTF.Copy)
        nc.sync.dma_start(
            out=out_hc[:, :].rearrange("(hc n) d -> n hc d", n=2), in_=out_sb[:, :, :]
        )

        if cfg.get("debug"):
            for nm, tl_, cast in [
                ("dbg_hsum3", hsum3B, None), ("dbg_fcsum3", fcsum3T, None),
                ("dbg_C3", C3_bf, None), ("dbg_fx3", fx3_nat, F32),
                ("dbg_hsum2", hsum2B, None), ("dbg_fcsum2", fcsum2T, None),
                ("dbg_C2", C2_bf, None), ("dbg_hsum1", hsum1B, None),
                ("dbg_fcsum1", fcsum1T, None), ("dbg_C1", C1T, None),
            ]:
                src = tl_[:, :, :]
                if cast is not None:
                    src = src.bitcast(cast)
                nc.sync.dma_start(out=dbg[nm][:, :, :], in_=src)

    nc.finalize()
    return nc


def _np_sigmoid(v):
    return 1.0 / (1.0 + np.exp(-v))


def _host_prep(x, wi_w, wo_w, wu_w, wf_w, wi_b, wo_b, wu_b, wf_b):
    import ml_dtypes

    E4 = ml_dtypes.float8_e4m3

    def q8(a):
        return np.asarray(a, np.float32).astype(E4)

    xs = np.asarray(x, np.float32).T  # [512, 69905]
    xt8_full = q8(xs * SX)
    xt8d_full = q8(xs)
    xt8l_full = q8(16.0 * (xs * SX - xt8_full.astype(np.float32)))

    def wpair(w, s):
        wT = np.ascontiguousarray(np.asarray(w, np.float32).T) * s
        hi = q8(wT)
        lo = q8(16.0 * (wT - hi.astype(np.float32)))
        return hi, lo

    common = {}
    for nm, w, s in [
        ("wix", wi_w[:, :512], SWX), ("wih", wi_w[:, 512:], SWH),
        ("wox", wo_w[:, :512], SWX), ("woh", wo_w[:, 512:], SWH),
        ("wux", wu_w[:, :512], SWX), ("wuh", wu_w[:, 512:], SWH),
        ("wfx", wf_w[:, :512], SWX), ("wfh", wf_w[:, 512:], SWX),
    ]:
        hi, lo = wpair(w, s)
        common[nm + "_h"] = hi
        common[nm + "_l"] = lo
    common["wux_d"] = q8(np.ascontiguousarray(np.asarray(wu_w[:, :512], np.float32).T) * (SWX / 16.0))
    common["wix_d"] = q8(np.ascontiguousarray(np.asarray(wi_w[:, :512], np.float32).T) * (SWX / 16.0))
    common["wox_d"] = q8(np.ascontiguousarray(np.asarray(wo_w[:, :512], np.float32).T) * (SWX / 16.0))
    common["wuh_d"] = q8(np.ascontiguousarray(np.asarray(wu_w[:, 512:], np.float32).T) * (SWH / 16.0))
    common["wfh_d"] = q8(np.ascontiguousarray(np.asarray(wf_w[:, 512:], np.float32).T) * (SWX / 16.0))
    common["wfx_d"] = q8(np.ascontiguousarray(np.asarray(wf_w[:, :512], np.float32).T) * (SWX / 16.0))

    def bias8(b):
        bs = np.asarray(b, np.float32).reshape(4, 128) * SWX
        hi = bs.astype(E4)
        lo = (16.0 * (bs - hi.astype(np.float32))).astype(E4)
        out = np.zeros((1, 1024), E4)
        ov = out[0, :].reshape(4, 2, 128)
        ov[:, 0, :] = hi
        ov[:, 1, :] = lo
        return out

    ones8 = np.zeros((1, 1024), np.float32)
    ones8[0, 0:512] = SX
    ones8[0, 512:1024] = 1.0
    common.update({
        "bias8i": bias8(wi_b), "bias8o": bias8(wo_b), "bias8u": bias8(wu_b),
        "ones8": ones8.astype(E4),
        "b4096": np.ascontiguousarray(np.asarray(wf_b, np.float32)[None, :] * SPS),
        "S": (np.arange(512)[None, :] // 16 == (np.arange(128) % 32)[:, None]).astype(np.float32),
        "S8": np.concatenate([
            64.0 * (np.arange(512)[None, :] // 16 == (np.arange(128) % 32)[:, None]),
            4.0 * (np.arange(512)[None, :] // 16 == (np.arange(128) % 32)[:, None]),
        ], axis=1).astype(np.float32).astype(E4),
        "onescol": np.ones((1, 128), np.float32),
    })
    in_maps = []
    for c in range(NC_N):
        def cols(full):
            return np.ascontiguousarray(np.concatenate(
                [
                    full[:, OFFS[4] + 8192 * c : OFFS[4] + 8192 * (c + 1)],
                    full[:, OFFS[3] + 512 * c : OFFS[3] + 512 * (c + 1)],
                    full[:, OFFS[2] + 32 * c : OFFS[2] + 32 * (c + 1)],
                    full[:, OFFS[1] + 2 * c : OFFS[1] + 2 * (c + 1)],
                ],
                axis=1,
            ))
        in_maps.append({"xt8": cols(xt8_full), "xt8d": cols(xt8d_full),
                        "xt8l": cols(xt8l_full), **common})
    return in_maps


def _host_finish(x, H1_all, C1_all, wi_w, wi_b, wf_w, wf_b, wo_w, wo_b, wu_w, wu_b):
    """Level 0 (root): its 16 children are the level-1 nodes across cores."""
    f8 = np.float64
    x0 = np.asarray(x[0], f8)
    H1 = np.asarray(H1_all, f8)
    C1 = np.asarray(C1_all, f8)
    hsum0 = H1.sum(0)
    f0 = _np_sigmoid(
        x0 @ np.asarray(wf_w, f8)[:, :512].T + C1 @ np.asarray(wf_w, f8)[:, 512:].T + np.asarray(wf_b, f8)
    )
    fcsum0 = (f0 * C1).sum(0)
    xh0 = np.concatenate([x0, hsum0])
    i0 = _np_sigmoid(xh0 @ np.asarray(wi_w, f8).T + np.asarray(wi_b, f8))
    o0 = _np_sigmoid(xh0 @ np.asarray(wo_w, f8).T + np.asarray(wo_b, f8))
    u0 = np.tanh(xh0 @ np.asarray(wu_w, f8).T + np.asarray(wu_b, f8))
    C0 = i0 * u0 + fcsum0
    H0 = o0 * np.tanh(C0)
    return H0.astype(np.float32), C0.astype(np.float32)


def _run(in_maps, trace=False):
    from concourse.bass_utils import run_bass_kernel_spmd

    if "nc" not in _CACHE:
        _CACHE["nc"] = _build_nc()
    return run_bass_kernel_spmd(_CACHE["nc"], in_maps, list(range(NC_N)), trace=trace)


def kernel(x, wi_w, wi_b, wf_w, wf_b, wo_w, wo_b, wu_w, wu_b, _trace=False):
    x = np.asarray(x, np.float32)
    in_maps = _host_prep(x, wi_w, wo_w, wu_w, wf_w, wi_b, wo_b, wu_b, wf_b)
    res = _run(in_maps, trace=_trace)
    _CACHE["last_results"] = res
    H1_all = np.concatenate([res.results[c]["out_hc"][0:2] for c in range(NC_N)])
    C1_all = np.concatenate([res.results[c]["out_hc"][2:4] for c in range(NC_N)])
    H0, C0 = _host_finish(x, H1_all, C1_all, wi_w, wi_b, wf_w, wf_b, wo_w, wo_b, wu_w, wu_b)
    return H0, C0


# revision 78
# speedup vs baseline: 1.1457x; 1.0503x over previous
"""CS-TreeLSTM (BRANCH=16, DEPTH=4, IN=HID=512) on 8 Trainium2 NeuronCores.

Strategy (data-parallel over subtrees, per the sharding hint):
  - Each core owns 8192 leaves, 512 level-3, 32 level-2, 2 level-1 nodes.
  - Activations live transposed on-chip: [hid/in on partitions, nodes on free].
  - Gate matmuls run as fp8-e4m3 DoubleRow (256-deep contraction per
    instruction at 0.5 cycles/row).  Weight quantization error is systematic
    across nodes and amplifies ~8x per tree level through the 16-child sums,
    so every weight is sent as a hi+lo fp8 PAIR (wl = fp8(16*(s*w - wh)));
    each gate runs a hi wave and a residual wave, giving ~1.3e-3 effective
    weight precision (2nd order in fp8 eps) at half the f32r PE cost.
    Activation (x, C, hsum) quantization is iid across nodes and cancels in
    the sibling sums, so single fp8 is enough for i/o/f; the u gate (tanh
    slope 1, feeds C directly) also gets an x-residual wave.
  - Scales: x8=16x, x8d=x, xl8=16*(16x-x8); Wx hi=256w; Wh hi=4096w;
    every i/o/u psum = 4096*pre, one merged ACT evac with scale 1/4096.
  - Gate biases ride tiny DoubleRow aug matmuls (hi slice x ones=16, lo
    residual slice x ones=1), making evacs single merged instructions (ACT is
    the bottleneck engine).
  - Forget gates: psum = Wfh_hi x C8(16C) + Wfh_lo x C8d(C) + f32r
    indicator-matrix aug carrying s*(Wfx x_parent + b_f) (computed once per
    level from an exact psum, weight pair included).
  - Sibling sums: h-path via bf16 pairwise-add tree (2x DVE mode) ->
    bf16 -> fp8 hi/lo; fcsum via plain reduce with f32 output.
  - Level 3 runs as two 256-node halves, the first interleaved at leaf chunk
    8 so its PE/DVE work hides under leaf-phase ACT; level 2 likewise as two
    16-node halves (chunk 10 / tail).
  - Level 0 (root; children span all cores) is combined on the host from
    per-core H1/C1 outputs (the only cross-core communication, 8x8KB).
"""

import sys

sys.path.insert(0, "/opt/trn_rl_repo")

import numpy as np

BRANCH = 16
DEPTH = 4
IN = 512
HID = 512
NC_N = 8
SIZES = [BRANCH**d for d in range(DEPTH + 1)]  # [1,16,256,4096,65536]
OFFS = [0, 1, 17, 273, 4369, 69905]
XT_COLS = 8192 + 512 + 32 + 2  # 8738
C3_OFF = 8192
C2_OFF = 8192 + 512
N_CHUNK = 16

SX = 16.0     # fp8 x hi scale
SWX = 256.0   # fp8 x-part / fh weight hi scale
SWH = 4096.0  # fp8 h-part weight hi scale
SPS = 4096.0  # i/o/u psum scale

_CACHE = {}

WNAMES = ["wix", "wih", "wox", "woh", "wux", "wuh", "wfx", "wfh"]


def _build_nc(cfg=None):
    cfg = cfg or {}
    from concourse import bacc
    import concourse.mybir as mybir
    import concourse.tile as tile
    from concourse.masks import make_identity

    F32 = mybir.dt.float32
    F32R = mybir.dt.float32r
    BF16 = mybir.dt.float16  # fp16: 10-bit mantissa, same DVE speed as bf16
    F8 = mybir.dt.float8e4
    ACTF = mybir.ActivationFunctionType
    AX = mybir.AxisListType
    OP = mybir.AluOpType
    DR = mybir.MatmulPerfMode.DoubleRow

    nc = bacc.Bacc()

    xt8 = nc.declare_dram_parameter("xt8", [IN, XT_COLS], F8, isOutput=False)
    xt8d = nc.declare_dram_parameter("xt8d", [IN, XT_COLS], F8, isOutput=False)
    xt8l = nc.declare_dram_parameter("xt8l", [IN, XT_COLS], F8, isOutput=False)
    wps = {}
    for n in WNAMES:
        wps[n + "_h"] = nc.declare_dram_parameter(n + "_h", [IN, HID], F8, isOutput=False)
        wps[n + "_l"] = nc.declare_dram_parameter(n + "_l", [IN, HID], F8, isOutput=False)
    for n in ("wux_d", "wix_d", "wox_d", "wuh_d", "wfh_d", "wfx_d"):
        wps[n] = nc.declare_dram_parameter(n, [IN, HID], F8, isOutput=False)
    bias8p = {g: nc.declare_dram_parameter("bias8" + g, [1, 1024], F8, isOutput=False) for g in "iou"}
    ones8p = nc.declare_dram_parameter("ones8", [1, 1024], F8, isOutput=False)
    b4096p = nc.declare_dram_parameter("b4096", [1, HID], F32, isOutput=False)
    S_p = nc.declare_dram_parameter("S", [128, 512], F32, isOutput=False)
    S8p = nc.declare_dram_parameter("S8", [128, 1024], F8, isOutput=False)
    onescolp = nc.declare_dram_parameter("onescol", [1, 128], F32, isOutput=False)
    out_hc = nc.declare_dram_parameter("out_hc", [4, HID], F32, isOutput=True)
    dbg = {}
    if cfg.get("debug"):
        for nm, shp, dt_ in [
            ("dbg_hsum3", [128, 4, 512], BF16), ("dbg_fcsum3", [128, 4, 512], F32),
            ("dbg_C3", [128, 4, 512], BF16), ("dbg_fx3", [128, 4, 512], F32),
            ("dbg_hsum2", [128, 4, 32], BF16), ("dbg_fcsum2", [128, 4, 32], F32),
            ("dbg_C2", [128, 4, 32], BF16), ("dbg_hsum1", [128, 4, 2], BF16),
            ("dbg_fcsum1", [128, 4, 2], F32), ("dbg_C1", [128, 4, 2], F32),
            ("dbg_C0ch", [128, 4, 512], BF16),
        ]:
            dbg[nm] = nc.declare_dram_parameter(nm, shp, dt_, isOutput=True)

    def t_view(h):  # DRAM [512, n] -> [128 part, 4 ktile, n] view
        return h[:, :].rearrange("(t p) n -> p t n", p=128)

    from contextlib import ExitStack

    with tile.TileContext(nc) as tc, ExitStack() as ctx:
        consts = ctx.enter_context(tc.tile_pool(name="consts", bufs=1))
        stream = ctx.enter_context(tc.tile_pool(name="stream", bufs=cfg.get("stream", 2)))
        workA = ctx.enter_context(tc.tile_pool(name="workA", bufs=2))
        workB = ctx.enter_context(tc.tile_pool(name="workB", bufs=2))
        longp = ctx.enter_context(tc.tile_pool(name="longp", bufs=1))
        psum = ctx.enter_context(tc.tile_pool(name="psum", bufs=4, space="PSUM"))

        LOWP = "paired-fp8/bf16 pipeline, tolerance 2e-2"

        # ---------------- constants / weights ----------------
        bias8 = {}
        for g in "iou":
            bias8[g] = consts.tile([1, 4, 2, 128], F8, tag="bias8" + g, name="bias8" + g)
            nc.sync.dma_start(
                out=bias8[g][:, :, :, :],
                in_=bias8p[g][:, :].rearrange("a (m two f) -> a m two f", two=2, f=128),
            )
        ones8 = consts.tile([1, 2, 512], F8, tag="ones8")
        nc.sync.dma_start(out=ones8[:, :, :], in_=ones8p[:, :].rearrange("a (two f) -> a two f", two=2))

        stream_tiles = {}

        def load_chunk(c):
            th = stream.tile([128, 4, 512], F8, tag="xt_c", name=f"xt_c{c}")
            nc.sync.dma_start(out=th[:, :, :], in_=t_view(xt8)[:, :, c * 512 : (c + 1) * 512])
            td = stream.tile([128, 4, 512], F8, tag="xt_d", name=f"xt_d{c}")
            nc.sync.dma_start(out=td[:, :, :], in_=t_view(xt8d)[:, :, c * 512 : (c + 1) * 512])
            tl = stream.tile([128, 4, 512], F8, tag="xt_l", name=f"xt_l{c}")
            nc.sync.dma_start(out=tl[:, :, :], in_=t_view(xt8l)[:, :, c * 512 : (c + 1) * 512])
            stream_tiles[c] = (th, td, tl)
            return stream_tiles[c]

        W = {}

        def load_w(n):
            W[n] = consts.tile([128, 4, HID], F8, tag="w_" + n, name="w_" + n)
            nc.sync.dma_start(out=W[n][:, :, :], in_=t_view(wps[n]))

        load_chunk(0)
        load_w("wix_h")
        load_w("wix_l")
        load_w("wox_h")
        load_w("wox_l")
        load_w("wux_h")
        load_w("wux_l")
        load_w("wux_d")
        load_w("wix_d")
        load_w("wox_d")
        load_w("wuh_d")
        load_chunk(1)
        xt3 = {}
        for sfx, src in (("h", xt8), ("d", xt8d), ("l", xt8l)):
            xt3[sfx] = consts.tile([128, 4, 512], F8, tag="xt3" + sfx, name="xt3" + sfx)
            nc.sync.dma_start(out=xt3[sfx][:, :, :], in_=t_view(src)[:, :, C3_OFF : C3_OFF + 512])
        load_w("wfx_h")
        load_w("wfx_l")
        load_w("wfx_d")
        onescol = consts.tile([1, 128], F32R, tag="onescol")
        nc.sync.dma_start(out=onescol[:, :], in_=onescolp[:, :].bitcast(F32R))
        b4096 = consts.tile([1, 512], F32R, tag="b4096")
        nc.sync.dma_start(out=b4096[:, :], in_=b4096p[:, :].bitcast(F32R))
        S_sb = consts.tile([128, 512], F32R, tag="S")
        nc.sync.dma_start(out=S_sb[:, :], in_=S_p[:, :].bitcast(F32R))
        S8_sb = consts.tile([128, 2, 512], F8, tag="S8")
        nc.sync.dma_start(out=S8_sb[:, :, :], in_=S8p[:, :].rearrange("p (two n) -> p two n", two=2))
        load_w("wfh_h")
        load_w("wfh_l")
        load_w("wfh_d")
        load_w("wih_h")
        load_w("wih_l")
        load_w("woh_h")
        load_w("woh_l")
        load_w("wuh_h")
        load_w("wuh_l")
        # tail x: [128,4,34] views for moving, clean-stride [128,4,32] for
        # fx stationaries (dual-fp8 Ldweights needs plane stride % 32 == 0)
        xtail = {}
        for sfx, src in (("h", xt8), ("d", xt8d), ("l", xt8l)):
            xtail[sfx] = consts.tile([128, 4, 34], F8, tag="xtail" + sfx, name="xtail" + sfx)
            nc.sync.dma_start(out=xtail[sfx][:, :, :], in_=t_view(src)[:, :, C2_OFF : C2_OFF + 34])
        xt2s = {}
        xt1s = {}
        for sfx, src in (("h", xt8), ("d", xt8d), ("l", xt8l)):
            xt2s[sfx] = consts.tile([128, 4, 32], F8, tag="xt2s" + sfx, name="xt2s" + sfx)
            nc.sync.dma_start(out=xt2s[sfx][:, :, :], in_=t_view(src)[:, :, C2_OFF : C2_OFF + 32])
            xt1s[sfx] = consts.tile([128, 4, 32], F8, tag="xt1s" + sfx, name="xt1s" + sfx)
            nc.sync.dma_start(out=xt1s[sfx][:, :, 0:2], in_=t_view(src)[:, :, C2_OFF + 32 : C2_OFF + 34])
        ident = consts.tile([128, 128], F32, tag="ident")
        make_identity(nc, ident[:, :])

        # persistent accumulators
        hsum3B = longp.tile([128, 4, 512], BF16, tag="hsum3B")
        hsum3 = {"h": longp.tile([128, 4, 512], F8, tag="hsum3h", name="hsum3h"),
                 "d": longp.tile([128, 4, 512], F8, tag="hsum3d", name="hsum3d")}
        hsum3res = longp.tile([128, 4, 512], F8, tag="hsum3res")
        hsT3 = longp.tile([128, 4, 32], BF16, tag="hsT3")
        fcsum3T = longp.tile([128, 4, 512], F32, tag="fcsum3T")
        fx3_nat = longp.tile([128, 4, 512], F32R, tag="fx3_nat")  # 4096*(fx3+bf)
        fx38 = longp.tile([128, 4, 2, 512], F8, tag="fx38")  # hi/lo pair at scale 64/16x
        # [0:32, 0, :] = 2048*(fx2+bf); [0:2, 1, :] = 256*(fx1+bf)
        fx21_nat = longp.tile([128, 2, 512], F32R, tag="fx21_nat")

        def dr_gate(g, rhs, n=512, hpart=None, col0=0, hcol0=None):
            """i/o/u gate psum: 4096*(W x (+ W h) + b) via paired-fp8 DR waves.
            rhs = (hi, d, l) x tiles; hpart = (hi8, d8) hsum tiles.
            u gets a 3rd x wave (x residual, tanh slope 1 feeds C directly)."""
            if hcol0 is None:
                hcol0 = col0
            qa = psum.tile([128, 2, 512], F32, tag="ps", name="ps_a")
            qb = psum.tile([128, 2, 512], F32, tag="ps", name="ps_b")
            ps = (qa, qb)
            for m in range(4):
                nc.tensor.matmul(
                    ps[m // 2][:, m % 2, :n], bias8[g][0:1, m, :, :], ones8[:, :, :n],
                    start=True, stop=False, perf_mode=DR,
                )
            if g == "o":
                waves = [("wox_h", rhs[0], col0), ("wox_l", rhs[1], col0)]
            else:
                waves = [("w" + g + "x_h", rhs[0], col0), ("w" + g + "x_l", rhs[1], col0),
                         ("w" + g + "x_d", rhs[2], col0)]
            if hpart is not None:
                waves.append(("w" + g + "h_h", hpart[0], hcol0))
                waves.append(("w" + g + "h_l", hpart[1], hcol0))
                if g == "u":
                    waves.append(("wuh_d", hpart[2], hcol0))
            for wi, (wn, rr, c0) in enumerate(waves):
                last_wave = wi == len(waves) - 1
                for m in range(4):
                    ms = slice(m * 128, (m + 1) * 128)
                    for kp in range(2):
                        nc.tensor.matmul(
                            ps[m // 2][:, m % 2, :n], W[wn][:, 2 * kp : 2 * kp + 2, ms],
                            rr[:, 2 * kp : 2 * kp + 2, c0 : c0 + n],
                            start=False, stop=(last_wave and kp == 1), perf_mode=DR,
                        )
            return ps

        def evac(ps, out_sb, act, scale, n=512):
            nc.scalar.activation(out_sb[:, 0:2, :n], ps[0][:, :, :n], act, scale=scale)
            nc.scalar.activation(out_sb[:, 2:4, :n], ps[1][:, :, :n], act, scale=scale)

        def f_gate_dr(C8, C8d, C8l, fx_sb, fx_rows, fx_col, n=512):
            """forget gate psum: f32r indicator aug (fx+bias) + paired fp8 DR on C."""
            qa = psum.tile([128, 2, 512], F32, tag="ps", name="ps_a")
            qb = psum.tile([128, 2, 512], F32, tag="ps", name="ps_b")
            ps = (qa, qb)
            b0, b1 = fx_rows
            for m in range(4):
                ms = slice(m * 128, (m + 1) * 128)
                nc.tensor.matmul(
                    ps[m // 2][:, m % 2, :n], fx_sb[b0:b1, fx_col, :, ms], S8_sb[b0:b1, :, :n],
                    start=True, stop=False, tile_position=(b0, 0), perf_mode=DR,
                )
                for wn, rr in (("wfh_h", C8), ("wfh_l", C8d), ("wfh_d", C8l)):
                    for kp in range(2):
                        nc.tensor.matmul(
                            ps[m // 2][:, m % 2, :n], W[wn][:, 2 * kp : 2 * kp + 2, ms],
                            rr[:, 2 * kp : 2 * kp + 2, :n],
                            start=False, stop=(wn == "wfh_d" and kp == 1), perf_mode=DR,
                        )
            return ps

        def fx3_compute():
            # fx3_nat = 4096*(x3 @ WfxT + bf)  (natural layout, nodes on partitions)
            qa = psum.tile([128, 2, 512], F32, tag="ps", name="ps_a")
            qb = psum.tile([128, 2, 512], F32, tag="ps", name="ps_b")
            psx = (qa, qb)
            for blk in range(4):
                pbs = slice(blk * 128, (blk + 1) * 128)
                sl_ = psx[blk // 2][:, blk % 2, :]
                nc.tensor.matmul(sl_, onescol[:, :], b4096[:, :], start=True, stop=False)
                for sfx, wn in (("h", "wfx_h"), ("d", "wfx_l"), ("l", "wfx_d")):
                    for kp in range(2):
                        nc.tensor.matmul(
                            sl_, xt3[sfx][:, 2 * kp : 2 * kp + 2, pbs],
                            W[wn][:, 2 * kp : 2 * kp + 2, :],
                            start=False, stop=(sfx == "l" and kp == 1), perf_mode=DR,
                        )
            fxT = workB.tile([128, 4, 512], BF16, tag="H", name="fxT")
            for half in range(2):
                hs = slice(2 * half, 2 * half + 2)
                nc.scalar.activation(fx3_nat[:, hs, :], psx[half][:, :, :], ACTF.Copy)
                nc.scalar.activation(fx38[:, hs, 0, :], psx[half][:, :, :], ACTF.Copy, scale=1 / 64.0)
                with nc.allow_low_precision(LOWP):
                    nc.vector.scalar_tensor_tensor(
                        fxT[:, hs, :], psx[half][:, :, :], 1 / 64.0, fx38[:, hs, 0, :],
                        op0=OP.mult, op1=OP.subtract,
                    )
            with nc.allow_low_precision(LOWP):
                nc.vector.tensor_scalar_mul(fx38[:, :, 1, :], fxT[:, :, :], 16.0)

        def fx21_compute():
            ps = psum.tile([128, 2, 512], F32, tag="ps", name="ps_a")
            nc.tensor.matmul(ps[0:32, 0, :], onescol[:, 0:32], b4096[:, :], start=True, stop=False)
            for sfx, wn in (("h", "wfx_h"), ("d", "wfx_l"), ("l", "wfx_d")):
                for kp in range(2):
                    nc.tensor.matmul(
                        ps[0:32, 0, :], xt2s[sfx][:, 2 * kp : 2 * kp + 2, :],
                        W[wn][:, 2 * kp : 2 * kp + 2, :],
                        start=False, stop=(sfx == "l" and kp == 1), perf_mode=DR,
                    )
            nc.tensor.matmul(ps[0:2, 1, :], onescol[:, 0:2], b4096[:, :], start=True, stop=False)
            for sfx, wn in (("h", "wfx_h"), ("d", "wfx_l"), ("l", "wfx_d")):
                for kp in range(2):
                    nc.tensor.matmul(
                        ps[0:2, 1, :], xt1s[sfx][:, 2 * kp : 2 * kp + 2, 0:2],
                        W[wn][:, 2 * kp : 2 * kp + 2, :],
                        start=False, stop=(sfx == "l" and kp == 1), perf_mode=DR,
                    )
            nc.scalar.activation(fx21_nat[0:32, 0, :], ps[0:32, 0, :], ACTF.Copy, scale=1 / 2.0)
            nc.scalar.activation(fx21_nat[0:2, 1, :], ps[0:2, 1, :], ACTF.Copy, scale=1 / 16.0)

        # ---------------- leaf phase ----------------
        def tree16(out_ap, src, pfx, g=32):
            # grouped sum of 16 along the free dim via bf16 pairwise adds (2x DVE mode)
            v = src[:, :, :].rearrange("p t (g w) -> p t g w", w=16)
            a1 = workB.tile([128, 4, g, 8], BF16, tag=f"tr1_{g}", name=pfx + "a1")
            a2 = workB.tile([128, 4, g, 4], BF16, tag=f"tr2_{g}", name=pfx + "a2")
            a3 = workB.tile([128, 4, g, 2], BF16, tag=f"tr3_{g}", name=pfx + "a3")
            nc.vector.tensor_add(a1[:, :, :, :], v[:, :, :, 0:8], v[:, :, :, 8:16])
            nc.vector.tensor_add(a2[:, :, :, :], a1[:, :, :, 0:4], a1[:, :, :, 4:8])
            nc.vector.tensor_add(a3[:, :, :, :], a2[:, :, :, 0:2], a2[:, :, :, 2:4])
            nc.vector.tensor_add(out_ap, a3[:, :, :, 0], a3[:, :, :, 1])

        def leaf_fpath(c, C_prev, C8_prev, C8d_prev, C8l_prev):
            b, pt = 32 * (c % 4), c // 4
            ps_f = f_gate_dr(C8_prev, C8d_prev, C8l_prev, fx38, (b, b + 32), pt)
            f_bf = workB.tile([128, 4, 512], BF16, tag="Ug")
            evac(ps_f, f_bf, ACTF.Sigmoid, 1 / SPS)
            fC = workB.tile([128, 4, 512], BF16, tag="H")
            with nc.allow_low_precision(LOWP):
                nc.vector.tensor_mul(fC[:, :, :], f_bf[:, :, :], C_prev[:, :, :])
                nc.vector.tensor_reduce(
                    fcsum3T[:, :, 32 * c : 32 * c + 32],
                    fC[:, :, :].rearrange("p t (g w) -> p t g w", w=16),
                    axis=AX.X, op=OP.add,
                )

        def leaf_hpath(c, C_bf, o_bf):
            tC = workA.tile([128, 4, 512], BF16, tag="A")
            nc.scalar.activation(tC[:, :, :], C_bf[:, :, :], ACTF.Tanh)
            H = workB.tile([128, 4, 512], BF16, tag="H")
            cols = slice(32 * c, 32 * c + 32)
            with nc.allow_low_precision(LOWP):
                nc.vector.tensor_mul(H[:, :, :], o_bf[:, :, :], tC[:, :, :])
                tree16(hsum3B[:, :, cols], H, f"hs{c}")
                nc.vector.tensor_scalar_mul(hsum3["h"][:, :, cols], hsum3B[:, :, cols], 1.0)
                nc.vector.tensor_scalar_mul(hsum3["d"][:, :, cols], hsum3B[:, :, cols], 1 / 16.0)
                nc.vector.tensor_sub(hsT3[:, :, :], hsum3B[:, :, cols], hsum3["h"][:, :, cols])
                nc.vector.tensor_scalar_mul(hsum3res[:, :, cols], hsT3[:, :, :], 16.0)

        # ---------------- level 3 (512 nodes, two 256-node halves) ----------------
        C3_bf = longp.tile([128, 4, 512], BF16, tag="C3_bf")
        C8_3 = longp.tile([128, 4, 512], F8, tag="C8_3")
        C8_3d = longp.tile([128, 4, 512], F8, tag="C8_3d")
        C3res = longp.tile([128, 4, 512], F8, tag="C3res")
        hsum2B = longp.tile([128, 4, 32], BF16, tag="hsum2B")
        hsum2 = {"h": longp.tile([128, 4, 32], F8, tag="hsum2h", name="hsum2h"),
                 "d": longp.tile([128, 4, 32], F8, tag="hsum2d", name="hsum2d")}
        hsum2res = longp.tile([128, 4, 32], F8, tag="hsum2res")
        hsT2 = longp.tile([128, 4, 16], BF16, tag="hsT2")
        fcsum2T = longp.tile([128, 4, 32], F32, tag="fcsum2T")

        def l3_half(h):
            sl = slice(256 * h, 256 * h + 256)
            g16 = slice(16 * h, 16 * h + 16)
            hp = (hsum3["h"], hsum3["d"], hsum3res)
            x3t = (xt3["h"], xt3["d"], xt3["l"])
            ps3 = dr_gate("i", x3t, n=256, hpart=hp, col0=256 * h)
            i3 = workA.tile([128, 4, 256], BF16, tag="A3", name=f"i3_{h}")
            evac(ps3, i3, ACTF.Sigmoid, 1 / SPS, n=256)
            ps3 = dr_gate("o", x3t, n=256, hpart=hp, col0=256 * h)
            o3 = workB.tile([128, 4, 256], BF16, tag="B3", name=f"o3_{h}")
            evac(ps3, o3, ACTF.Sigmoid, 1 / SPS, n=256)
            ps3 = dr_gate("u", x3t, n=256, hpart=hp, col0=256 * h)
            u3 = workB.tile([128, 4, 256], BF16, tag="U3", name=f"u3_{h}")
            evac(ps3, u3, ACTF.Tanh, 1 / SPS, n=256)

            iu3 = workB.tile([128, 4, 256], BF16, tag="H3", name=f"iu3_{h}")
            C3t = workB.tile([128, 4, 256], BF16, tag="B3", name=f"C3t_{h}")
            with nc.allow_low_precision(LOWP):
                nc.vector.tensor_mul(iu3[:, :, :], i3[:, :, :], u3[:, :, :])
                nc.vector.tensor_add(C3_bf[:, :, sl], iu3[:, :, :], fcsum3T[:, :, sl])
                nc.vector.tensor_scalar_mul(C8_3[:, :, sl], C3_bf[:, :, sl], 8.0)
                nc.gpsimd.tensor_scalar_mul(C8_3d[:, :, sl], C3_bf[:, :, sl], 0.5)
                # C3res = 16*(8*C3 - C8_3): f2's C residual wave
                nc.vector.scalar_tensor_tensor(
                    C3t[:, :, :], C3_bf[:, :, sl], 8.0, C8_3[:, :, sl],
                    op0=OP.mult, op1=OP.subtract,
                )
                nc.gpsimd.tensor_scalar_mul(C3res[:, :, sl], C3t[:, :, :], 16.0)
            tC3 = workA.tile([128, 4, 256], BF16, tag="A3", name=f"tC3_{h}")
            nc.scalar.activation(tC3[:, :, :], C3_bf[:, :, sl], ACTF.Tanh)
            H3 = workB.tile([128, 4, 256], BF16, tag="H3", name=f"H3_{h}")
            with nc.allow_low_precision(LOWP):
                nc.vector.tensor_mul(H3[:, :, :], o3[:, :, :], tC3[:, :, :])
                tree16(hsum2B[:, :, g16], H3, f"hs2_{h}", g=16)
                nc.vector.tensor_scalar_mul(hsum2["h"][:, :, g16], hsum2B[:, :, g16], 1.0)
                nc.vector.tensor_scalar_mul(hsum2["d"][:, :, g16], hsum2B[:, :, g16], 1 / 16.0)
                nc.vector.tensor_sub(hsT2[:, :, :], hsum2B[:, :, g16], hsum2["h"][:, :, g16])
                nc.vector.tensor_scalar_mul(hsum2res[:, :, g16], hsT2[:, :, :], 16.0)

            # f-path to level 2 for this half: psum = 2048*pre_f2
            # wave1: wfh_h(256w) x C8_3(8*C3); wave2: wfh_l(16r) x C8_3d(C3/2)
            qa2 = psum.tile([128, 2, 512], F32, tag="ps", name="ps_a")
            qb2 = psum.tile([128, 2, 512], F32, tag="ps", name="ps_b")
            ps_f2 = (qa2, qb2)
            for m in range(4):
                ms = slice(m * 128, (m + 1) * 128)
                sl_ = ps_f2[m // 2][:, m % 2, :256]
                nc.tensor.matmul(
                    sl_, fx21_nat[0:32, 0, ms], S_sb[0:32, sl],
                    start=True, stop=False, tile_position=(0, 0),
                )
                for wn, rr in (("wfh_h", C8_3), ("wfh_l", C8_3d), ("wfh_d", C3res)):
                    for kp in range(2):
                        nc.tensor.matmul(
                            sl_, W[wn][:, 2 * kp : 2 * kp + 2, ms],
                            rr[:, 2 * kp : 2 * kp + 2, sl],
                            start=False, stop=(wn == "wfh_d" and kp == 1), perf_mode=DR,
                        )
            f2 = workB.tile([128, 4, 256], BF16, tag="U3", name=f"f2_{h}")
            evac(ps_f2, f2, ACTF.Sigmoid, 1 / 2048.0, n=256)
            fC2 = workB.tile([128, 4, 256], BF16, tag="B3", name=f"fC2_{h}")
            with nc.allow_low_precision(LOWP):
                nc.vector.tensor_mul(fC2[:, :, :], f2[:, :, :], C3_bf[:, :, sl])
                nc.vector.tensor_reduce(
                    fcsum2T[:, :, g16],
                    fC2[:, :, :].rearrange("p t (g w) -> p t g w", w=16),
                    axis=AX.X, op=OP.add,
                )

        # ---------------- level 2 (32 nodes, two 16-node halves) ----------------
        C2_bf = longp.tile([128, 4, 32], BF16, tag="C2_bf")
        C2_d = longp.tile([128, 4, 32], BF16, tag="C2_d")
        hsum1B = longp.tile([128, 4, 2], BF16, tag="hsum1B")
        hsum1 = {"h": longp.tile([128, 4, 2], F8, tag="hsum1h", name="hsum1h"),
                 "d": longp.tile([128, 4, 2], F8, tag="hsum1d", name="hsum1d")}
        hsum1res = longp.tile([128, 4, 2], F8, tag="hsum1res")
        hsT1 = longp.tile([128, 4, 1], BF16, tag="hsT1")
        fcsum1T = longp.tile([128, 4, 2], F32, tag="fcsum1T")

        def l2_half(h):
            g16 = slice(16 * h, 16 * h + 16)
            g1 = slice(h, h + 1)
            hp = (hsum2["h"], hsum2["d"], hsum2res)
            xtt = (xtail["h"], xtail["d"], xtail["l"])
            ps2 = dr_gate("i", xtt, n=16, hpart=hp, col0=16 * h)
            i2 = longp.tile([128, 4, 16], BF16, tag="s_i", name=f"i2_{h}")
            evac(ps2, i2, ACTF.Sigmoid, 1 / SPS, n=16)
            ps2 = dr_gate("o", xtt, n=16, hpart=hp, col0=16 * h)
            o2 = longp.tile([128, 4, 16], BF16, tag="s_o", name=f"o2_{h}")
            evac(ps2, o2, ACTF.Sigmoid, 1 / SPS, n=16)
            ps2 = dr_gate("u", xtt, n=16, hpart=hp, col0=16 * h)
            u2 = longp.tile([128, 4, 16], BF16, tag="s_u", name=f"u2_{h}")
            evac(ps2, u2, ACTF.Tanh, 1 / SPS, n=16)

            iu2 = longp.tile([128, 4, 16], BF16, tag="s_t", name=f"iu2_{h}")
            tC2 = longp.tile([128, 4, 16], BF16, tag="s_t2", name=f"tC2_{h}")
            H2 = longp.tile([128, 4, 16], BF16, tag="s_h", name=f"H2_{h}")
            with nc.allow_low_precision(LOWP):
                nc.vector.tensor_mul(iu2[:, :, :], i2[:, :, :], u2[:, :, :])
                nc.vector.tensor_add(C2_bf[:, :, g16], iu2[:, :, :], fcsum2T[:, :, g16])
                nc.vector.tensor_scalar_mul(C2_d[:, :, g16], C2_bf[:, :, g16], 1 / 16.0)
            nc.scalar.activation(tC2[:, :, :], C2_bf[:, :, g16], ACTF.Tanh)
            with nc.allow_low_precision(LOWP):
                nc.vector.tensor_mul(H2[:, :, :], o2[:, :, :], tC2[:, :, :])
                nc.vector.tensor_reduce(
                    hsum1B[:, :, g1],
                    H2[:, :, :].rearrange("p t (g w) -> p t g w", w=16),
                    axis=AX.X, op=OP.add,
                )
                nc.vector.tensor_scalar_mul(hsum1["h"][:, :, g1], hsum1B[:, :, g1], 1.0)
                nc.vector.tensor_scalar_mul(hsum1["d"][:, :, g1], hsum1B[:, :, g1], 1 / 16.0)
                nc.vector.tensor_sub(hsT1[:, :, :], hsum1B[:, :, g1], hsum1["h"][:, :, g1])
                nc.vector.tensor_scalar_mul(hsum1res[:, :, g1], hsT1[:, :, :], 16.0)

            # f-path to level 1 for this half: psum = 256*pre_f1
            # wave1: wfh_h(256w) x C2_bf (bf16); wave2: wfh_l(16r) x C2_d(C2/16)
            qa1 = psum.tile([128, 2, 512], F32, tag="ps", name="ps_a")
            qb1 = psum.tile([128, 2, 512], F32, tag="ps", name="ps_b")
            ps_f1 = (qa1, qb1)
            for m in range(4):
                ms = slice(m * 128, (m + 1) * 128)
                sl_ = ps_f1[m // 2][:, m % 2, 0:16]
                nc.tensor.matmul(
                    sl_, fx21_nat[0:2, 1, ms], S_sb[0:2, g16],
                    start=True, stop=False, tile_position=(0, 0),
                )
                for wn, rr in (("wfh_h", C2_bf), ("wfh_l", C2_d)):
                    for k in range(4):
                        nc.tensor.matmul(
                            sl_, W[wn][:, k, ms], rr[:, k, g16],
                            start=False, stop=(wn == "wfh_l" and k == 3),
                        )
            f1 = longp.tile([128, 4, 16], BF16, tag="s_f1", name=f"f1_{h}")
            evac(ps_f1, f1, ACTF.Sigmoid, 1 / 256.0, n=16)
            fC1 = longp.tile([128, 4, 16], F32, tag="s_fc1", name=f"fC1_{h}")
            with nc.allow_low_precision(LOWP):
                nc.vector.tensor_mul(fC1[:, :, :], f1[:, :, :], C2_bf[:, :, g16])
            nc.vector.tensor_reduce(
                fcsum1T[:, :, g1],
                fC1[:, :, :].rearrange("p t (g w) -> p t g w", w=16),
                axis=AX.X, op=OP.add,
            )

        Ct = longp.tile([128, 4, 512], BF16, tag="Ct")
        n_chunk = cfg.get("n_chunk", N_CHUNK)
        pipe = None  # (chunk index, C_bf, C8, C8d)
        for c in range(n_chunk):
            xt_c = stream_tiles.pop(c) if c in stream_tiles else load_chunk(c)
            if c + 1 < n_chunk and (c + 1) not in stream_tiles:
                load_chunk(c + 1)

            ps_i = dr_gate("i", xt_c)
            i_bf = workA.tile([128, 4, 512], BF16, tag="A")
            evac(ps_i, i_bf, ACTF.Sigmoid, 1 / SPS)

            ps_o = dr_gate("o", xt_c)
            o_bf = workB.tile([128, 4, 512], BF16, tag="B")
            evac(ps_o, o_bf, ACTF.Sigmoid, 1 / SPS)

            ps_u = dr_gate("u", xt_c)
            u_bf = workB.tile([128, 4, 512], BF16, tag="Ug")
            evac(ps_u, u_bf, ACTF.Tanh, 1 / SPS)

            C_bf = workA.tile([128, 4, 512], BF16, tag="C")
            C8 = workA.tile([128, 4, 512], F8, tag="C8")
            C8d = workA.tile([128, 4, 512], F8, tag="C8d")
            C8l = workA.tile([128, 4, 512], F8, tag="C8l")
            with nc.allow_low_precision(LOWP):
                nc.vector.tensor_mul(C_bf[:, :, :], i_bf[:, :, :], u_bf[:, :, :])
                nc.vector.tensor_scalar_mul(C8[:, :, :], C_bf[:, :, :], SX)
                nc.gpsimd.tensor_scalar_mul(C8d[:, :, :], C_bf[:, :, :], 1.0)
                # C8l = 16*(16*C - C8): the f-gate's C residual wave
                nc.vector.scalar_tensor_tensor(
                    Ct[:, :, :], C_bf[:, :, :], SX, C8[:, :, :],
                    op0=OP.mult, op1=OP.subtract,
                )
                nc.gpsimd.tensor_scalar_mul(C8l[:, :, :], Ct[:, :, :], 16.0)

            if c == 0:
                fx3_compute()
                if cfg.get("debug"):
                    nc.sync.dma_start(out=dbg["dbg_C0ch"][:, :, :], in_=C_bf[:, :, :])
            if c == 1:
                fx21_compute()
            if pipe is not None:
                leaf_fpath(*pipe)

            leaf_hpath(c, C_bf, o_bf)
            pipe = (c, C_bf, C8, C8d, C8l)

            if c == 8:
                l3_half(0)
            if c == 10:
                l2_half(0)

        leaf_fpath(*pipe)
        l3_half(1)
        l2_half(1)

        # ---------------- level 1 (2 nodes, transposed) ----------------
        x1v = (xtail["h"], xtail["d"], xtail["l"])
        hp1 = (hsum1["h"], hsum1["d"], hsum1res)
        ps1 = dr_gate("i", x1v, n=2, hpart=hp1, col0=32, hcol0=0)
        i1 = longp.tile([128, 4, 2], F32, tag="s_i1")
        evac(ps1, i1, ACTF.Sigmoid, 1 / SPS, n=2)
        ps1 = dr_gate("o", x1v, n=2, hpart=hp1, col0=32, hcol0=0)
        o1 = longp.tile([128, 4, 2], F32, tag="s_o1")
        evac(ps1, o1, ACTF.Sigmoid, 1 / SPS, n=2)
        ps1 = dr_gate("u", x1v, n=2, hpart=hp1, col0=32, hcol0=0)
        u1 = longp.tile([128, 4, 2], F32, tag="s_u1")
        evac(ps1, u1, ACTF.Tanh, 1 / SPS, n=2)

        iu1 = longp.tile([128, 4, 2], F32, tag="s_iu1")
        C1T = longp.tile([128, 4, 2], F32, tag="C1T")
        nc.vector.tensor_mul(iu1[:, :, :], i1[:, :, :], u1[:, :, :])
        nc.vector.tensor_add(C1T[:, :, :], iu1[:, :, :], fcsum1T[:, :, :])
        tC1 = longp.tile([128, 4, 2], F32, tag="s_tc1")
        nc.scalar.activation(tC1[:, :, :], C1T[:, :, :], ACTF.Tanh)
        H1T = longp.tile([128, 4, 2], F32, tag="s_h1")
        nc.vector.tensor_mul(H1T[:, :, :], o1[:, :, :], tC1[:, :, :])

        # transpose to natural layout and DMA out
        ps_t = psum.tile([128, 2, 512], F32, tag="ps", name="ps_a")
        for t in range(4):
            nc.tensor.transpose(ps_t[0:2, 0, t * 128 : (t + 1) * 128], H1T[:, t, :], ident[:, :])
            nc.tensor.transpose(ps_t[0:2, 1, t * 128 : (t + 1) * 128], C1T[:, t, :], ident[:, :])
        out_sb = longp.tile([2, 2, 512], F32, tag="out_sb")
        nc.scalar.activation(out_sb[:, :, :], ps_t[0:2, 0:2, :], ACTF.Copy)
        nc.sync.dma_start(
            out=out_hc[:, :].rearrange("(hc n) d -> n hc d", n=2), in_=out_sb[:, :, :]
        )

        if cfg.get("debug"):
            for nm, tl_, cast in [
                ("dbg_hsum3", hsum3B, None), ("dbg_fcsum3", fcsum3T, None),
                ("dbg_C3", C3_bf, None), ("dbg_fx3", fx3_nat, F32),
                ("dbg_hsum2", hsum2B, None), ("dbg_fcsum2", fcsum2T, None),
                ("dbg_C2", C2_bf, None), ("dbg_hsum1", hsum1B, None),
                ("dbg_fcsum1", fcsum1T, None), ("dbg_C1", C1T, None),
            ]:
                src = tl_[:, :, :]
                if cast is not None:
                    src = src.bitcast(cast)
                nc.sync.dma_start(out=dbg[nm][:, :, :], in_=src)

    nc.finalize()
    return nc


def _np_sigmoid(v):
    return 1.0 / (1.0 + np.exp(-v))


def _host_prep(x, wi_w, wo_w, wu_w, wf_w, wi_b, wo_b, wu_b, wf_b):
    import ml_dtypes

    E4 = ml_dtypes.float8_e4m3

    def q8(a):
        return np.asarray(a, np.float32).astype(E4)

    xs = np.asarray(x, np.float32).T  # [512, 69905]
    xt8_full = q8(xs * SX)
    xt8d_full = q8(xs)
    xt8l_full = q8(16.0 * (xs * SX - xt8_full.astype(np.float32)))

    def wpair(w, s):
        wT = np.ascontiguousarray(np.asarray(w, np.float32).T) * s
        hi = q8(wT)
        lo = q8(16.0 * (wT - hi.astype(np.float32)))
        return hi, lo

    common = {}
    for nm, w, s in [
        ("wix", wi_w[:, :512], SWX), ("wih", wi_w[:, 512:], SWH),
        ("wox", wo_w[:, :512], SWX), ("woh", wo_w[:, 512:], SWH),
        ("wux", wu_w[:, :512], SWX), ("wuh", wu_w[:, 512:], SWH),
        ("wfx", wf_w[:, :512], SWX), ("wfh", wf_w[:, 512:], SWX),
    ]:
        hi, lo = wpair(w, s)
        common[nm + "_h"] = hi
        common[nm + "_l"] = lo
    common["wux_d"] = q8(np.ascontiguousarray(np.asarray(wu_w[:, :512], np.float32).T) * (SWX / 16.0))
    common["wix_d"] = q8(np.ascontiguousarray(np.asarray(wi_w[:, :512], np.float32).T) * (SWX / 16.0))
    common["wox_d"] = q8(np.ascontiguousarray(np.asarray(wo_w[:, :512], np.float32).T) * (SWX / 16.0))
    common["wuh_d"] = q8(np.ascontiguousarray(np.asarray(wu_w[:, 512:], np.float32).T) * (SWH / 16.0))
    common["wfh_d"] = q8(np.ascontiguousarray(np.asarray(wf_w[:, 512:], np.float32).T) * (SWX / 16.0))
    common["wfx_d"] = q8(np.ascontiguousarray(np.asarray(wf_w[:, :512], np.float32).T) * (SWX / 16.0))

    def bias8(b):
        bs = np.asarray(b, np.float32).reshape(4, 128) * SWX
        hi = bs.astype(E4)
        lo = (16.0 * (bs - hi.astype(np.float32))).astype(E4)
        out = np.zeros((1, 1024), E4)
        ov = out[0, :].reshape(4, 2, 128)
        ov[:, 0, :] = hi
        ov[:, 1, :] = lo
        return out

    ones8 = np.zeros((1, 1024), np.float32)
    ones8[0, 0:512] = SX
    ones8[0, 512:1024] = 1.0
    common.update({
        "bias8i": bias8(wi_b), "bias8o": bias8(wo_b), "bias8u": bias8(wu_b),
        "ones8": ones8.astype(E4),
        "b4096": np.ascontiguousarray(np.asarray(wf_b, np.float32)[None, :] * SPS),
        "S": (np.arange(512)[None, :] // 16 == (np.arange(128) % 32)[:, None]).astype(np.float32),
        "S8": np.concatenate([
            64.0 * (np.arange(512)[None, :] // 16 == (np.arange(128) % 32)[:, None]),
            4.0 * (np.arange(512)[None, :] // 16 == (np.arange(128) % 32)[:, None]),
        ], axis=1).astype(np.float32).astype(E4),
        "onescol": np.ones((1, 128), np.float32),
    })
    in_maps = []
    for c in range(NC_N):
        def cols(full):
            return np.ascontiguousarray(np.concatenate(
                [
                    full[:, OFFS[4] + 8192 * c : OFFS[4] + 8192 * (c + 1)],
                    full[:, OFFS[3] + 512 * c : OFFS[3] + 512 * (c + 1)],
                    full[:, OFFS[2] + 32 * c : OFFS[2] + 32 * (c + 1)],
                    full[:, OFFS[1] + 2 * c : OFFS[1] + 2 * (c + 1)],
                ],
                axis=1,
            ))
        in_maps.append({"xt8": cols(xt8_full), "xt8d": cols(xt8d_full),
                        "xt8l": cols(xt8l_full), **common})
    return in_maps


def _host_finish(x, H1_all, C1_all, wi_w, wi_b, wf_w, wf_b, wo_w, wo_b, wu_w, wu_b):
    """Level 0 (root): its 16 children are the level-1 nodes across cores."""
    f8 = np.float64
    x0 = np.asarray(x[0], f8)
    H1 = np.asarray(H1_all, f8)
    C1 = np.asarray(C1_all, f8)
    hsum0 = H1.sum(0)
    f0 = _np_sigmoid(
        x0 @ np.asarray(wf_w, f8)[:, :512].T + C1 @ np.asarray(wf_w, f8)[:, 512:].T + np.asarray(wf_b, f8)
    )
    fcsum0 = (f0 * C1).sum(0)
    xh0 = np.concatenate([x0, hsum0])
    i0 = _np_sigmoid(xh0 @ np.asarray(wi_w, f8).T + np.asarray(wi_b, f8))
    o0 = _np_sigmoid(xh0 @ np.asarray(wo_w, f8).T + np.asarray(wo_b, f8))
    u0 = np.tanh(xh0 @ np.asarray(wu_w, f8).T + np.asarray(wu_b, f8))
    C0 = i0 * u0 + fcsum0
    H0 = o0 * np.tanh(C0)
    return H0.astype(np.float32), C0.astype(np.float32)


def _run(in_maps, trace=False):
    from concourse.bass_utils import run_bass_kernel_spmd

    if "nc" not in _CACHE:
        _CACHE["nc"] = _build_nc()
    return run_bass_kernel_spmd(_CACHE["nc"], in_maps, list(range(NC_N)), trace=trace)


def kernel(x, wi_w, wi_b, wf_w, wf_b, wo_w, wo_b, wu_w, wu_b, _trace=False):
    x = np.asarray(x, np.float32)
    in_maps = _host_prep(x, wi_w, wo_w, wu_w, wf_w, wi_b, wo_b, wu_b, wf_b)
    res = _run(in_maps, trace=_trace)
    _CACHE["last_results"] = res
    H1_all = np.concatenate([res.results[c]["out_hc"][0:2] for c in range(NC_N)])
    C1_all = np.concatenate([res.results[c]["out_hc"][2:4] for c in range(NC_N)])
    H0, C0 = _host_finish(x, H1_all, C1_all, wi_w, wi_b, wf_w, wf_b, wo_w, wo_b, wu_w, wu_b)
    return H0, C0
